# revision 28
# baseline (speedup 1.0000x reference)
"""Conformer layer on 8 Trainium2 NeuronCores (v2).

Sharding: core c handles batch b=c//2. Within a batch pair:
 - token-parallel (halves of T=1024) for FFN1/conv/FFN2/LN stages,
 - head-parallel (4 heads each) for attention.

v2 changes vs baseline:
 - LayerNorm fold: matmuls consume raw activations; per-token scale/shift is
   applied as a post-matmul fixup z = psy*rec + colsum(W')*(-m*rec) + b', so
   the PE never waits for LN statistics (keeps the HAM clock warm).
 - bf16 operands for attention (q/k/v/p/probs/o_h) with moving-dim-1024
   matmuls, bf16 FFN/pw/wo weights (same PE rate, half the DMA).
 - AllGather carries bf16 h and overlaps with p-projection + local-half QKV;
   a warmup collective at kernel start absorbs the first-cc latency.
 - Attention ReduceScatter in bf16.
 - Depthwise-conv diagonal matrices built on-chip (saves an 8MB DMA).
 - Plain loads on HWDGE (nc.sync), freeing GpSimd for casts/broadcasts.
"""

import numpy as np

import concourse.bass as bass
import concourse.mybir as mybir
import concourse.tile as tile
from concourse import bacc
from concourse.bass import ds, ts
from concourse.bass_utils import run_bass_kernel_spmd
from contextlib import ExitStack

F32 = mybir.dt.float32
F32R = mybir.dt.float32r
BF16 = mybir.dt.bfloat16
AF = mybir.ActivationFunctionType
ALU = mybir.AluOpType

D, DFF, H, DK, KCONV = 512, 2048, 8, 64, 31
B, T = 4, 1024
EPS = 1e-5
HT = 512            # tokens per core
WIN = 544           # conv window: 16 + 512 + 16
PB = 2047
BAND = 1152         # bd band width per q-chunk
BST = 1160          # bd dram row stride (elements)
NCORES = 8

PAIRS = [[0, 1], [2, 3], [4, 5], [6, 7]]
ALLG = [[0, 1, 2, 3, 4, 5, 6, 7]]


def _r(ap):
    return ap.bitcast(F32R)


def _emit(nc):
    def inp(name, shape, dt=F32):
        return nc.dram_tensor(name, list(shape), dt, kind="ExternalInput")

    x_d = inp("x_loc", (4, 128, HT), F32R)
    xbf_d = inp("x_bf", (4, 128, HT), BF16)
    wf1_d = inp("wf1", (4, 128, DFF), BF16)
    sf1_d = inp("sf1", (128, 16)); bf1_d = inp("bf1", (128, 16))
    wf2_d = inp("wf2", (16, 128, D), BF16); bf2_d = inp("bf2", (128, 4))
    wq_d = inp("wq", (4, 128, 256), BF16)
    sq_d = inp("sq", (128, 2)); bq_d = inp("bq", (128, 2))
    dqv_d = inp("dqv", (128, 2))
    wk_d = inp("wk", (4, 128, 256), BF16)
    sk_d = inp("sk", (128, 2)); bk_d = inp("bk", (128, 2))
    wv_d = inp("wv", (4, 128, 256), BF16)
    sv_d = inp("svrow", (1, 256)); bv_d = inp("bvrow", (1, 256))
    wp_d = inp("wp", (4, 128, 256), BF16)
    wo_d = inp("wo", (4, 64, D), BF16); bo_d = inp("bo", (128, 4))
    posT_d = inp("posT", (4, 128, 2048), BF16)
    pw1_d = inp("pw1", (4, 128, 1024), BF16)
    spw_d = inp("spw", (128, 8)); bpw1_d = inp("bpw1", (128, 8))
    dwv_d = inp("dwv", (128, 4, KCONV))
    bng_d = inp("bng", (128, 4)); bnb_d = inp("bnb", (128, 4))
    pw2_d = inp("pw2", (4, 128, D), BF16); bpw2_d = inp("bpw2", (128, 4))
    cmask_d = inp("cmask", (1, WIN))
    wg1_d = inp("wg1", (4, 128, DFF), BF16)
    sg1_d = inp("sg1", (128, 16)); bg1_d = inp("bg1", (128, 16))
    wg2_d = inp("wg2", (16, 128, D), BF16); bg2_d = inp("bg2", (128, 4))
    g5_d = inp("g5", (128, 4)); b5_d = inp("b5", (128, 4))
    onc_d = inp("onc", (128, 1), F32R)
    oncb_d = inp("onc_bf", (128, 1), BF16)
    idn_d = inp("idn", (128, 128), F32R)

    out_d = nc.dram_tensor("out_loc", [4, 128, HT], F32, kind="ExternalOutput")

    cc_w_in = nc.dram_tensor("cc_w_in", [1, 64], F32)
    cc_w_out = nc.dram_tensor("cc_w_out", [2, 64], F32)
    cc_h_in = nc.dram_tensor("cc_h_in", [4, 128, HT], BF16)
    cc_h_out = nc.dram_tensor("cc_h_out", [2, 4, 128, HT], BF16)
    cc_a_in = nc.dram_tensor("cc_a_in", [2, 4, 128, WIN], BF16)
    cc_a_out = nc.dram_tensor("cc_a_out", [4, 128, WIN], BF16)
    cc_bn_in = nc.dram_tensor("cc_bn_in", [128, 8], F32)
    cc_bn_out = nc.dram_tensor("cc_bn_out", [128, 8], F32)
    bd_d = [nc.dram_tensor(f"bd_{i}", [128 * BST], BF16) for i in range(32)]

    uid = [0]

    with tile.TileContext(nc) as tc, ExitStack() as ctx:
        const = ctx.enter_context(tc.tile_pool(name="const", bufs=1))
        ones_c = const.tile([128, 1], F32R)
        nc.sync.dma_start(out=ones_c[:], in_=onc_d[:])
        ones_cb = const.tile([128, 1], BF16)
        nc.sync.dma_start(out=ones_cb[:], in_=oncb_d[:])
        eps1 = const.tile([1, 1], F32); nc.vector.memset(eps1[:], EPS)
        epsP = const.tile([128, 1], F32); nc.vector.memset(epsP[:], EPS)
        ident = const.tile([128, 128], F32R)
        nc.sync.dma_start(out=ident[:], in_=idn_d[:])
        identb = const.tile([128, 128], BF16)
        nc.vector.tensor_copy(identb[:], ident[:].bitcast(F32))

        # CC-stream warmup: tiny pair AllGather, result unused.
        warm = const.tile([1, 64], F32)
        nc.vector.memset(warm[:], 0.0)
        nc.sync.dma_start(out=cc_w_in[:], in_=warm[:])
        nc.gpsimd.collective_compute(
            "AllGather", ALU.bypass, ins=[cc_w_in[:]], outs=[cc_w_out[:]],
            replica_groups=PAIRS)

        # per-engine copies of the core's token-half index (register values
        # are engine-local)
        pidv = nc.vector.partition_id()
        scv, scv2 = pidv % 2, (pidv + 1) % 2
        pida = nc.scalar.partition_id()
        sca, sca2 = pida % 2, (pida + 1) % 2
        pids = nc.sync.partition_id()
        scs, scs2 = pids % 2, (pids + 1) % 2

        act = ctx.enter_context(tc.tile_pool(name="act", bufs=1))
        hfe_sb = act.tile([128, 4, WIN], F32R)   # conv residual window
        h_sb = act.tile([128, 4, HT], F32R)      # post-FFN1 hidden (local)
        h_bf = act.tile([128, 4, HT], BF16)
        h3_sb = act.tile([128, 4, HT], F32R)     # post-conv hidden
        h3_bf = act.tile([128, 4, HT], BF16)
        h4_sb = act.tile([128, 4, HT], F32R)     # post-FFN2 hidden

        # ---------- LN statistics (fold form) ----------
        # Produces broadcast tiles RB = 1/std and NM = -mean/std per token.
        def emit_stats(x4, nchunk, W, blocks, rb_t, nb_t, col0, sbp, ones):
            uid[0] += 1
            with tc.tile_pool(name=f"lnps{uid[0]}", bufs=1,
                              space="PSUM") as lnps:
                x2 = sbp.tile([128, nchunk, W], F32R, tag="ln_sq")
                nc.vector.tensor_mul(x2[:], x4, x4)
                for b0, bw in blocks:
                    pss = lnps.tile([1, bw], F32, tag="lns")
                    psq = lnps.tile([1, bw], F32, tag="lnq")
                    for c in range(nchunk):
                        nc.tensor.matmul(pss[:], ones,
                                         x4[:, c, b0:b0 + bw],
                                         start=(c == 0), stop=(c == nchunk - 1))
                    for c in range(nchunk):
                        nc.tensor.matmul(psq[:], _r(ones_c[:]),
                                         _r(x2[:, c, b0:b0 + bw]),
                                         start=(c == 0), stop=(c == nchunk - 1))
                    mng = sbp.tile([1, bw], F32, tag="ln_m")
                    nc.scalar.activation(mng[:], pss[:], AF.Copy,
                                         scale=-1.0 / D)
                    e2 = sbp.tile([1, bw], F32, tag="ln_e2")
                    nc.scalar.activation(e2[:], psq[:], AF.Copy, scale=1.0 / D)
                    var = sbp.tile([1, bw], F32, tag="ln_var")
                    nc.vector.tensor_mul(var[:], mng[:], mng[:])
                    nc.vector.tensor_sub(var[:], e2[:], var[:])
                    sd = sbp.tile([1, bw], F32, tag="ln_sd")
                    nc.scalar.activation(sd[:], var[:], AF.Sqrt, bias=eps1[:])
                    rec = sbp.tile([1, bw], F32, tag="ln_rs")
                    scr = sbp.tile([1, bw], F32, tag="ln_scr")
                    nc.vector.reciprocal_approx_accurate(rec[:], sd[:], scr[:])
                    nmr = sbp.tile([1, bw], F32, tag="ln_nm")
                    nc.vector.tensor_mul(nmr[:], mng[:], rec[:])
                    nc.gpsimd.partition_broadcast(
                        rb_t[:, col0 + b0:col0 + b0 + bw], rec[:])
                    nc.gpsimd.partition_broadcast(
                        nb_t[:, col0 + b0:col0 + b0 + bw], nmr[:])

        # ---------- FFN with LN fold ----------
        def emit_ffn(xbf, xres, rb_t, nb_t, out, outbf, w1dram, s1dram,
                     b1dram, w2dram, b2dram, pref):
            with tc.tile_pool(name=pref + "w", bufs=1) as wp_, \
                 tc.tile_pool(name=pref + "t", bufs=3) as tp, \
                 tc.tile_pool(name=pref + "ps", bufs=2, space="PSUM") as psp, \
                 tc.tile_pool(name=pref + "ph", bufs=1, space="PSUM") as php:
                w1 = wp_.tile([128, 4, DFF], BF16)
                for c in range(4):
                    nc.sync.dma_start(out=w1[:, c, :], in_=w1dram[c])
                w2 = wp_.tile([128, 16, D], BF16)
                for j in range(16):
                    nc.sync.dma_start(out=w2[:, j, :], in_=w2dram[j])
                s1 = wp_.tile([128, 16], F32)
                nc.sync.dma_start(out=s1[:], in_=s1dram[:])
                b1 = wp_.tile([128, 16], F32)
                nc.sync.dma_start(out=b1[:], in_=b1dram[:])
                b2 = wp_.tile([128, 4], F32)
                nc.sync.dma_start(out=b2[:], in_=b2dram[:])
                psh = php.tile([128, 4, HT], F32)
                for j in range(16):
                    psy = psp.tile([128, HT], F32, tag="psy")
                    for c in range(4):
                        nc.tensor.matmul(psy[:], w1[:, c, ts(j, 128)],
                                         xbf[:, c, :],
                                         start=(c == 0), stop=(c == 3))
                    u = tp.tile([128, HT], F32, tag="u")
                    nc.vector.tensor_mul(u[:], psy[:], rb_t[:])
                    z = tp.tile([128, HT], F32, tag="z")
                    nc.vector.scalar_tensor_tensor(
                        out=z[:], in0=nb_t[:], scalar=s1[:, j:j + 1],
                        in1=u[:], op0=ALU.mult, op1=ALU.add)
                    sig = tp.tile([128, HT], F32, tag="sig")
                    nc.scalar.activation(sig[:], z[:], AF.Sigmoid,
                                         bias=b1[:, j:j + 1])
                    y1 = tp.tile([128, HT], BF16, tag="y1")
                    nc.vector.scalar_tensor_tensor(
                        out=y1[:], in0=z[:], scalar=b1[:, j:j + 1],
                        in1=sig[:], op0=ALU.add, op1=ALU.mult)
                    for f in range(4):
                        nc.tensor.matmul(psh[:, f, :],
                                         w2[:, j, ts(f, 128)], y1[:],
                                         start=(j == 0), stop=(j == 15))
                for c in range(4):
                    nc.vector.scalar_tensor_tensor(
                        out=out[:, c, :], in0=psh[:, c, :],
                        scalar=b2[:, c:c + 1], in1=xres[:, c, :],
                        op0=ALU.add, op1=ALU.add)
                    if outbf is not None:
                        nc.vector.tensor_copy(outbf[:, c, :],
                                              out[:, c, :].bitcast(F32))

        # ================= Stage A: FFN1 on local tokens =================
        with tc.tile_pool(name="stA", bufs=1) as stA, \
             tc.tile_pool(name="stAt", bufs=2) as stAt:
            x_sb = stA.tile([128, 4, HT], F32R)
            for c in range(4):
                nc.sync.dma_start(out=x_sb[:, c, :], in_=x_d[c])
            x_bf = stA.tile([128, 4, HT], BF16)
            for c in range(4):
                nc.sync.dma_start(out=x_bf[:, c, :], in_=xbf_d[c])
            rb1 = stA.tile([128, HT], F32)
            nb1 = stA.tile([128, HT], F32)
            emit_stats(x_sb[:], 4, HT, [(0, HT)], rb1, nb1, 0, stAt,
                       _r(ones_c[:]))
            emit_ffn(x_bf[:], x_sb[:], rb1, nb1, h_sb[:], h_bf[:],
                     wf1_d, sf1_d, bf1_d, wf2_d, bf2_d, "f1")
            # conv residual window: local half occupies [16, 528)
            for c in range(4):
                nc.vector.tensor_copy(hfe_sb[:, c, 16:16 + HT].bitcast(F32),
                                      h_sb[:, c, :].bitcast(F32))
            for c in range(4):
                nc.sync.dma_start(out=cc_h_in[c], in_=h_bf[:, c, :])

        nc.gpsimd.collective_compute(
            "AllGather", ALU.bypass, ins=[cc_h_in[:]], outs=[cc_h_out[:]],
            replica_groups=PAIRS)

        # ============ attention scope ============
        with tc.tile_pool(name="attp", bufs=1) as attp:
            woff = sca * HT                # local half start (ACT offsets)
            roff = sca2 * HT               # remote half start

            q_sb = attp.tile([128, 2, T], BF16)
            qv_sb = attp.tile([128, 2, T], BF16)
            k_sb = attp.tile([128, 2, T], BF16)
            v65 = attp.tile([128, 8, 4, 65], BF16)
            nc.vector.memset(v65[:, :, :, 64:65], 1.0)
            p_sb = attp.tile([128, 2, 2048], BF16)
            # rb2/nb2 and recc/nmc are stored local-half-first (static
            # writes); the eviction target offsets stay symbolic.
            rb2 = attp.tile([128, T], F32)
            nb2 = attp.tile([128, T], F32)
            recc = attp.tile([128, 8], F32)
            nmc = attp.tile([128, 8], F32)

            # ---- p-projection (overlaps AllGather) ----
            with tc.tile_pool(name="ppp", bufs=2, space="PSUM") as ppp, \
                 tc.tile_pool(name="atw", bufs=1) as atw:
                wp_sb = atw.tile([128, 4, 256], BF16, tag="wp")
                for c in range(4):
                    nc.sync.dma_start(out=wp_sb[:, c, :], in_=wp_d[c])
                pos_sb = atw.tile([128, 4, 2048], BF16, tag="pos")
                for c in range(4):
                    nc.sync.dma_start(out=pos_sb[:, c, :], in_=posT_d[c])
                for m in range(2):
                    for pc in range(4):
                        psp_t = ppp.tile([128, 512], F32, tag="psp")
                        for c in range(4):
                            nc.tensor.matmul(
                                psp_t[:], wp_sb[:, c, ts(m, 128)],
                                pos_sb[:, c, ts(pc, 512)],
                                start=(c == 0), stop=(c == 3))
                        nc.scalar.activation(p_sb[:, m, ts(pc, 512)],
                                             psp_t[:], AF.Copy)

            # ---- local-half LN2 stats + qkv (overlap AllGather) ----
            with tc.tile_pool(name="stC", bufs=1) as stC, \
                 tc.tile_pool(name="stCt", bufs=2) as stCt, \
                 tc.tile_pool(name="qkp", bufs=2, space="PSUM") as qkps:

                def emit_stats2(x4, ones, rcol):
                    # LN2 stats for one half; writes rb2/nb2[:, rcol:rcol+HT]
                    # and recc/nmc[:, rcol//128 : +4] (static columns).
                    uid[0] += 1
                    cc = rcol // 128
                    with tc.tile_pool(name=f"lnps{uid[0]}", bufs=1,
                                      space="PSUM") as lnps:
                        x2 = stCt.tile([128, 4, HT], F32R, tag="ln_sq")
                        nc.vector.tensor_mul(x2[:], x4, x4)
                        pss = lnps.tile([1, HT], F32, tag="lns")
                        psq = lnps.tile([1, HT], F32, tag="lnq")
                        for c in range(4):
                            nc.tensor.matmul(pss[:], ones, x4[:, c, :],
                                             start=(c == 0), stop=(c == 3))
                        for c in range(4):
                            nc.tensor.matmul(psq[:], _r(ones_c[:]),
                                             _r(x2[:, c, :]),
                                             start=(c == 0), stop=(c == 3))
                        mng = stCt.tile([1, HT], F32, tag="ln_m")
                        nc.scalar.activation(mng[:], pss[:], AF.Copy,
                                             scale=-1.0 / D)
                        e2 = stCt.tile([1, HT], F32, tag="ln_e2")
                        nc.scalar.activation(e2[:], psq[:], AF.Copy,
                                             scale=1.0 / D)
                        var = stCt.tile([1, HT], F32, tag="ln_var")
                        nc.vector.tensor_mul(var[:], mng[:], mng[:])
                        nc.vector.tensor_sub(var[:], e2[:], var[:])
                        sd = stCt.tile([1, HT], F32, tag="ln_sd")
                        nc.scalar.activation(sd[:], var[:], AF.Sqrt,
                                             bias=eps1[:])
                        rec2 = stCt.tile([1, HT], F32, tag="ln_rec")
                        scr = stCt.tile([1, HT], F32, tag="ln_scr")
                        nc.vector.reciprocal_approx_accurate(rec2[:], sd[:],
                                                             scr[:])
                        nm2 = stCt.tile([1, HT], F32, tag="ln_nm")
                        nc.vector.tensor_mul(nm2[:], mng[:], rec2[:])
                        nc.gpsimd.partition_broadcast(
                            rb2[:, rcol:rcol + HT], rec2[:])
                        nc.gpsimd.partition_broadcast(
                            nb2[:, rcol:rcol + HT], nm2[:])
                        for b in range(4):
                            nc.sync.dma_start(
                                out=recc[:, cc + b:cc + b + 1],
                                in_=rec2[:, ts(b, 128)])
                            nc.sync.dma_start(
                                out=nmc[:, cc + b:cc + b + 1],
                                in_=nm2[:, ts(b, 128)])

                wq_sb = stC.tile([128, 4, 256], BF16)
                wk_sb = stC.tile([128, 4, 256], BF16)
                wv_sb = stC.tile([128, 4, 256], BF16)
                for c in range(4):
                    nc.sync.dma_start(out=wq_sb[:, c, :], in_=wq_d[c])
                    nc.sync.dma_start(out=wk_sb[:, c, :], in_=wk_d[c])
                    nc.sync.dma_start(out=wv_sb[:, c, :], in_=wv_d[c])
                sq_sb = stC.tile([128, 2], F32)
                nc.sync.dma_start(out=sq_sb[:], in_=sq_d[:])
                bq_sb = stC.tile([128, 2], F32)
                nc.sync.dma_start(out=bq_sb[:], in_=bq_d[:])
                sk_sb = stC.tile([128, 2], F32)
                nc.sync.dma_start(out=sk_sb[:], in_=sk_d[:])
                bk_sb = stC.tile([128, 2], F32)
                nc.sync.dma_start(out=bk_sb[:], in_=bk_d[:])
                bqv_sb = stC.tile([128, 2], F32)
                nc.sync.dma_start(out=bqv_sb[:], in_=dqv_d[:])
                sv_row = stC.tile([1, 256], F32)
                nc.sync.dma_start(out=sv_row[:], in_=sv_d[:])
                svb = stC.tile([128, 256], F32)
                nc.gpsimd.partition_broadcast(svb[:], sv_row[:])
                bv_row = stC.tile([1, 256], F32)
                nc.sync.dma_start(out=bv_row[:], in_=bv_d[:])
                bvb = stC.tile([128, 256], F32)
                nc.gpsimd.partition_broadcast(bvb[:], bv_row[:])

                def emit_qk(xin, rcol, off):
                    # rcol: static column base in rb2/nb2 (local-first);
                    # off: symbolic global token offset for the outputs.
                    for m in range(2):
                        for w_sb, s_w, b_w, dst in (
                                (wq_sb, sq_sb, bq_sb, q_sb),
                                (wk_sb, sk_sb, bk_sb, k_sb)):
                            ps = qkps.tile([128, HT], F32, tag="psqk")
                            for c in range(4):
                                nc.tensor.matmul(
                                    ps[:], w_sb[:, c, ts(m, 128)],
                                    xin[:, c, :],
                                    start=(c == 0), stop=(c == 3))
                            u = stCt.tile([128, HT], F32, tag="qk_u")
                            nc.vector.tensor_mul(u[:], ps[:],
                                                 rb2[:, rcol:rcol + HT])
                            w_ = stCt.tile([128, HT], F32, tag="qk_w")
                            nc.vector.scalar_tensor_tensor(
                                out=w_[:], in0=nb2[:, rcol:rcol + HT],
                                scalar=s_w[:, m:m + 1], in1=u[:],
                                op0=ALU.mult, op1=ALU.add)
                            nc.scalar.activation(
                                dst[:, m, ds(off, HT)], w_[:], AF.Identity,
                                bias=b_w[:, m:m + 1])
                            if dst is q_sb:
                                nc.scalar.activation(
                                    qv_sb[:, m, ds(off, HT)], w_[:],
                                    AF.Identity, bias=bqv_sb[:, m:m + 1])

                def emit_v(xin, vbase, rbase):
                    # vbase: symbolic global chunk base; rbase: static
                    # column base into recc/nmc.
                    for tq in range(4):
                        psv = qkps.tile([128, 256], F32, tag="psv")
                        for c in range(4):
                            nc.tensor.matmul(
                                psv[:], xin[:, c, ts(tq, 128)],
                                wv_sb[:, c, :],
                                start=(c == 0), stop=(c == 3))
                        u = stCt.tile([128, 256], F32, tag="v_u")
                        nc.vector.tensor_scalar_mul(
                            u[:], psv[:], recc[:, rbase + tq:rbase + tq + 1])
                        w_ = stCt.tile([128, 256], F32, tag="v_w")
                        nc.vector.scalar_tensor_tensor(
                            out=w_[:], in0=svb[:],
                            scalar=nmc[:, rbase + tq:rbase + tq + 1],
                            in1=u[:], op0=ALU.mult, op1=ALU.add)
                        vt = stCt.tile([128, 256], BF16, tag="v_t")
                        nc.vector.tensor_add(vt[:], w_[:], bvb[:])
                        nc.vector.tensor_copy(
                            v65[:, ds(vbase + tq, 1), :, 0:64]
                            .rearrange("p o h d -> p (o h) d"),
                            vt[:].rearrange("p (h d) -> p h d", h=4))

                emit_stats2(h_sb[:], _r(ones_c[:]), 0)
                emit_qk(h_bf[:], 0, woff)
                emit_v(h_bf[:], scv * 4, 0)

                # ---- gather lands: remote half ----
                h_rem = stC.tile([128, 4, HT], BF16)
                for c in range(4):
                    nc.sync.dma_start(
                        out=h_rem[:, c, :],
                        in_=cc_h_out[:].rearrange("s c p t -> (s c) p t")
                        [ds(scs2 * 4 + c, 1)]
                        .rearrange("o p t -> (o p) t"))
                # conv halo: zero edges then copy 16 remote tokens
                nc.vector.memset(hfe_sb[:, :, 0:16].bitcast(F32), 0.0)
                nc.vector.memset(hfe_sb[:, :, 16 + HT:].bitcast(F32), 0.0)
                for c in range(4):
                    nc.vector.tensor_copy(
                        hfe_sb[:, c, ds(scv2 * (16 + HT), 16)].bitcast(F32),
                        h_rem[:, c, ds(scv * (HT - 16), 16)])

                emit_stats2(h_rem[:], ones_cb[:], HT)
                emit_qk(h_rem[:], HT, roff)
                emit_v(h_rem[:], scv2 * 4, 4)

            # ---- bd banded matmuls for all heads ----
            with tc.tile_pool(name="bds", bufs=2) as bdp, \
                 tc.tile_pool(name="psb", bufs=2, space="PSUM") as psb:
                for h in range(4):
                    hc, hr = h // 2, 64 * (h % 2)
                    for qc in range(8):
                        base = max(0, 895 - 128 * qc)
                        bdw = bdp.tile([128, BAND], BF16, tag="bdw")
                        for pi in range(3):
                            psB = psb.tile([128, 384], F32, tag="psB")
                            nc.tensor.matmul(
                                psB[:],
                                qv_sb[hr:hr + 64, hc, ts(qc, 128)],
                                p_sb[hr:hr + 64, hc,
                                     base + pi * 384: base + (pi + 1) * 384],
                                start=True, stop=True)
                            nc.vector.tensor_copy(
                                bdw[:, pi * 384:(pi + 1) * 384], psB[:])
                        nc.sync.dma_start(
                            out=bd_d[h * 8 + qc][:].rearrange(
                                "(p w) -> p w", p=128)[:, 0:BAND],
                            in_=bdw[:])

            # ---- attention heads: scores + softmax + AV ----
            o_h = [attp.tile([64, T], BF16, name=f"o_h{_h}", tag=f"o_h{_h}")
                   for _h in range(4)]
            with tc.tile_pool(name="bdsh", bufs=1) as shp, \
                 tc.tile_pool(name="atp", bufs=3) as atp, \
                 tc.tile_pool(name="atn", bufs=1) as atn, \
                 tc.tile_pool(name="pss", bufs=2, space="PSUM") as pss_p, \
                 tc.tile_pool(name="psav", bufs=2, space="PSUM") as psav:
                for h in range(4):
                    hc, hr = h // 2, 64 * (h % 2)
                    bdsh = shp.tile([128, 8, T], F32R, tag=f"sh{h % 2}")
                    for qc in range(8):
                        c0 = 127 if qc == 7 else 128
                        src = bass.AP(tensor=bd_d[h * 8 + qc], offset=c0,
                                      ap=[[BST - 1, 128], [1, T]])
                        nc.gpsimd.dma_start(out=bdsh[:, qc, :], in_=src)
                    psA = psav.tile([65, T], F32, tag="psAV")
                    for kc in range(8):
                        psS = pss_p.tile([128, T], F32, tag="psS")
                        for th in range(2):
                            nc.tensor.matmul(
                                psS[:, th * 512:(th + 1) * 512],
                                k_sb[hr:hr + 64, hc, ts(kc, 128)],
                                q_sb[hr:hr + 64, hc,
                                     th * 512:(th + 1) * 512],
                                start=True, stop=False, skip_group_check=True)
                        for qc in range(8):
                            nc.tensor.matmul(
                                _r(psS[:, ts(qc, 128)]),
                                _r(bdsh[:, qc, ts(kc, 128)]), _r(ident[:]),
                                is_transpose=True, start=False,
                                stop=(qc == 7), skip_group_check=True)
                        probs = atp.tile([128, T], BF16, tag="probs")
                        nc.scalar.activation(probs[:], psS[:], AF.Exp)
                        for th in range(2):
                            nc.tensor.matmul(
                                psA[:, th * 512:(th + 1) * 512],
                                v65[:, kc, h, :],
                                probs[:, th * 512:(th + 1) * 512],
                                start=(kc == 0), stop=(kc == 7),
                                skip_group_check=True)
                    s65 = atn.tile([65, T], F32, tag="s65")
                    nc.vector.tensor_copy(s65[64:65, :], psA[64:65, :])
                    row = atn.tile([1, T], F32, tag="row")
                    nc.gpsimd.dma_start(out=row[:], in_=s65[64:65, :])
                    rec = atn.tile([1, T], F32, tag="rec")
                    scr = atn.tile([1, T], F32, tag="scr")
                    nc.vector.reciprocal_approx_accurate(rec[:], row[:],
                                                         scr[:])
                    rb = atn.tile([64, T], F32, tag="rb")
                    nc.gpsimd.partition_broadcast(rb[:], rec[:])
                    nc.vector.tensor_mul(o_h[h][:], psA[0:64, :], rb[:])

            # ---- out-projection + pair ReduceScatter (bf16) ----
            with tc.tile_pool(name="pso", bufs=2, space="PSUM") as psop, \
                 tc.tile_pool(name="aot", bufs=2) as aot:
                wo_sb = aot.tile([64, 4, D], BF16, tag="wo", bufs=1)
                for hh in range(4):
                    nc.sync.dma_start(out=wo_sb[:, hh, :], in_=wo_d[hh])
                for f in range(4):
                    pso = psop.tile([128, T], F32, tag="pso")
                    for th in range(2):
                        for hh in range(4):
                            nc.tensor.matmul(
                                pso[:, th * 512:(th + 1) * 512],
                                wo_sb[:, hh, ts(f, 128)],
                                o_h[hh][:, th * 512:(th + 1) * 512],
                                start=(hh == 0), stop=(hh == 3),
                                skip_group_check=True)
                    ao = aot.tile([128, 2, WIN], BF16, tag="ao")
                    nc.vector.memset(ao[:, 0, 0:16], 0.0)
                    nc.vector.memset(ao[:, 1, WIN - 16:WIN], 0.0)
                    nc.vector.tensor_copy(ao[:, 0, 16:WIN], pso[:, 0:528])
                    nc.vector.tensor_copy(ao[:, 1, 0:528], pso[:, 496:T])
                    nc.sync.dma_start(out=cc_a_in[0, f], in_=ao[:, 0, :])
                    nc.sync.dma_start(out=cc_a_in[1, f], in_=ao[:, 1, :])

        nc.gpsimd.collective_compute(
            "ReduceScatter", ALU.add, ins=[cc_a_in[:]], outs=[cc_a_out[:]],
            replica_groups=PAIRS)

        # ================= Stage F: conv module =================
        with tc.tile_pool(name="stF", bufs=1) as stF, \
             tc.tile_pool(name="stFt", bufs=2) as stFt:
            # on-chip depthwise diag build (overlaps ReduceScatter)
            dwv = stF.tile([128, 4, KCONV], F32)
            nc.sync.dma_start(out=dwv[:], in_=dwv_d[:])
            dg = stF.tile([128, 4, KCONV, 128], BF16)
            for c in range(4):
                for j in range(KCONV):
                    nc.vector.tensor_scalar(
                        dg[:, c, j, :], identb[:],
                        dwv[:, c, j:j + 1], 0.0, ALU.mult, ALU.add)
            w1 = stF.tile([128, 4, 1024], BF16)
            for c in range(4):
                nc.sync.dma_start(out=w1[:, c, :], in_=pw1_d[c])
            spw = stF.tile([128, 8], F32)
            nc.sync.dma_start(out=spw[:], in_=spw_d[:])
            bp1 = stF.tile([128, 8], F32)
            nc.sync.dma_start(out=bp1[:], in_=bpw1_d[:])
            w2 = stF.tile([128, 4, D], BF16)
            for c in range(4):
                nc.sync.dma_start(out=w2[:, c, :], in_=pw2_d[c])
            bp2 = stF.tile([128, 4], F32)
            nc.sync.dma_start(out=bp2[:], in_=bpw2_d[:])
            bo_sb = stF.tile([128, 4], F32)
            nc.sync.dma_start(out=bo_sb[:], in_=bo_d[:])
            cm = stF.tile([1, WIN], F32)
            nc.sync.dma_start(out=cm[:], in_=cmask_d[:])
            cmb = stF.tile([128, WIN], F32)
            nc.gpsimd.partition_broadcast(cmb[:], cm[:])

            h2w = stF.tile([128, 4, WIN], F32R)
            for c in range(4):
                at = stFt.tile([128, WIN], F32, tag="at")
                nc.gpsimd.dma_start(out=at[:], in_=cc_a_out[c])
                nc.vector.scalar_tensor_tensor(
                    out=h2w[:, c, :], in0=at[:],
                    scalar=bo_sb[:, c:c + 1],
                    in1=hfe_sb[:, c, :].bitcast(F32),
                    op0=ALU.add, op1=ALU.add)
            h2w_bf = stF.tile([128, 4, WIN], BF16)
            for c in range(4):
                nc.vector.tensor_copy(h2w_bf[:, c, :],
                                      h2w[:, c, :].bitcast(F32))

            rb3 = stF.tile([128, WIN], F32)
            nb3 = stF.tile([128, WIN], F32)
            emit_stats(h2w[:], 4, WIN, [(0, 272), (272, 272)], rb3, nb3, 0,
                       stFt, _r(ones_c[:]))

            glu = stF.tile([128, 4, WIN], BF16)
            with tc.tile_pool(name="cvp1", bufs=1, space="PSUM") as cps:
                for m in range(4):
                    psa = cps.tile([128, 2, 512], F32, tag="psa")
                    psg = cps.tile([128, 2, 512], F32, tag="psg")
                    for half in range(2):
                        sl = slice(half * 272, (half + 1) * 272)
                        for c in range(4):
                            nc.tensor.matmul(psa[:, half, 0:272],
                                             w1[:, c, ts(m, 128)],
                                             h2w_bf[:, c, sl],
                                             start=(c == 0), stop=(c == 3),
                                             skip_group_check=True)
                        for c in range(4):
                            nc.tensor.matmul(psg[:, half, 0:272],
                                             w1[:, c, 512 + m * 128:
                                                 512 + (m + 1) * 128],
                                             h2w_bf[:, c, sl],
                                             start=(c == 0), stop=(c == 3),
                                             skip_group_check=True)
                    for half in range(2):
                        sl = slice(half * 272, (half + 1) * 272)
                        ua = stFt.tile([128, 272], F32, tag="cv_ua")
                        nc.vector.tensor_mul(ua[:], psa[:, half, 0:272],
                                             rb3[:, sl])
                        za = stFt.tile([128, 272], F32, tag="cv_za")
                        nc.vector.scalar_tensor_tensor(
                            out=za[:], in0=nb3[:, sl],
                            scalar=spw[:, m:m + 1], in1=ua[:],
                            op0=ALU.mult, op1=ALU.add)
                        ug = stFt.tile([128, 272], F32, tag="cv_ug")
                        nc.vector.tensor_mul(ug[:], psg[:, half, 0:272],
                                             rb3[:, sl])
                        zg = stFt.tile([128, 272], F32, tag="cv_zg")
                        nc.vector.scalar_tensor_tensor(
                            out=zg[:], in0=nb3[:, sl],
                            scalar=spw[:, 4 + m:5 + m], in1=ug[:],
                            op0=ALU.mult, op1=ALU.add)
                        sg = stFt.tile([128, 272], F32, tag="cv_sg")
                        nc.scalar.activation(sg[:], zg[:], AF.Sigmoid,
                                             bias=bp1[:, 4 + m:5 + m])
                        gl = stFt.tile([128, 272], F32, tag="cv_gl")
                        nc.vector.scalar_tensor_tensor(
                            out=gl[:], in0=za[:], scalar=bp1[:, m:m + 1],
                            in1=sg[:], op0=ALU.add, op1=ALU.mult)
                        nc.vector.tensor_mul(glu[:, m, sl], gl[:],
                                             cmb[:, sl])

            # depthwise conv: 31 accumulating diagonal matmuls per chunk
            acc = stF.tile([128, 4, HT], F32)
            with tc.tile_pool(name="dgp", bufs=2, space="PSUM") as dgp:
                for c in range(4):
                    psC = dgp.tile([128, HT], F32, tag="psC")
                    for j in range(KCONV):
                        nc.tensor.matmul(psC[:], dg[:, c, j, :],
                                         glu[:, c, 1 + j:1 + j + HT],
                                         start=(j == 0), stop=(j == KCONV - 1))
                    nc.vector.tensor_copy(acc[:, c, :], psC[:])
            # BN stats + 8-way AllReduce
            bnpack = stF.tile([128, 8], F32)
            for c in range(4):
                bst_t = stFt.tile([128, 6], F32, tag="bst")
                nc.vector.bn_stats(bst_t[:], acc[:, c, :])
                mv = stFt.tile([128, 2], F32, tag="mv")
                nc.vector.bn_aggr(mv[:], bst_t[:])
                nc.vector.tensor_copy(bnpack[:, 2 * c:2 * c + 1], mv[:, 0:1])
                nc.vector.scalar_tensor_tensor(
                    out=bnpack[:, 2 * c + 1:2 * c + 2], in0=mv[:, 0:1],
                    scalar=mv[:, 0:1], in1=mv[:, 1:2],
                    op0=ALU.mult, op1=ALU.add)
            nc.sync.dma_start(out=cc_bn_in[:], in_=bnpack[:])
            nc.gpsimd.collective_compute(
                "AllReduce", ALU.add, ins=[cc_bn_in[:]], outs=[cc_bn_out[:]],
                replica_groups=ALLG)
            bnar = stF.tile([128, 8], F32)
            nc.sync.dma_start(out=bnar[:], in_=cc_bn_out[:])
            bng_sb = stF.tile([128, 4], F32)
            nc.sync.dma_start(out=bng_sb[:], in_=bng_d[:])
            bnb_sb = stF.tile([128, 4], F32)
            nc.sync.dma_start(out=bnb_sb[:], in_=bnb_d[:])
            ysl = stF.tile([128, 4, HT], BF16)
            for c in range(4):
                mg = stFt.tile([128, 1], F32, tag="mg")
                nc.scalar.activation(mg[:], bnar[:, 2 * c:2 * c + 1], AF.Copy,
                                     scale=1.0 / NCORES)
                e2 = stFt.tile([128, 1], F32, tag="e2c")
                nc.scalar.activation(e2[:], bnar[:, 2 * c + 1:2 * c + 2],
                                     AF.Copy, scale=1.0 / NCORES)
                vg = stFt.tile([128, 1], F32, tag="vg")
                nc.vector.tensor_mul(vg[:], mg[:], mg[:])
                nc.vector.tensor_sub(vg[:], e2[:], vg[:])
                sdc = stFt.tile([128, 1], F32, tag="sdc")
                nc.scalar.activation(sdc[:], vg[:], AF.Sqrt, bias=epsP[:])
                rs = stFt.tile([128, 1], F32, tag="rsc")
                nc.vector.reciprocal(rs[:], sdc[:])
                s1 = stFt.tile([128, 1], F32, tag="s1c")
                nc.vector.tensor_mul(s1[:], rs[:], bng_sb[:, c:c + 1])
                s2 = stFt.tile([128, 1], F32, tag="s2c")
                nc.vector.tensor_mul(s2[:], mg[:], s1[:])
                nc.vector.tensor_sub(s2[:], bnb_sb[:, c:c + 1], s2[:])
                sg2 = stFt.tile([128, HT], F32, tag="sg2")
                nc.scalar.activation(sg2[:], acc[:, c, :], AF.Sigmoid,
                                     scale=s1[:], bias=s2[:])
                yt = stFt.tile([128, HT], F32, tag="yt")
                nc.vector.tensor_scalar(yt[:], acc[:, c, :],
                                        s1[:], s2[:], ALU.mult, ALU.add)
                nc.vector.tensor_mul(ysl[:, c, :], yt[:], sg2[:])
            with tc.tile_pool(name="cvp2", bufs=2, space="PSUM") as cps2:
                for f in range(4):
                    psw = cps2.tile([128, HT], F32, tag="psw")
                    for c in range(4):
                        nc.tensor.matmul(psw[:], w2[:, c, ts(f, 128)],
                                         ysl[:, c, :],
                                         start=(c == 0), stop=(c == 3))
                    nc.vector.scalar_tensor_tensor(
                        out=h3_sb[:, f, :], in0=psw[:], scalar=bp2[:, f:f + 1],
                        in1=h2w[:, f, 16:16 + HT], op0=ALU.add, op1=ALU.add)
                    nc.vector.tensor_copy(h3_bf[:, f, :],
                                          h3_sb[:, f, :].bitcast(F32))

        # ================= Stage G: FFN2 =================
        with tc.tile_pool(name="stG", bufs=1) as stG, \
             tc.tile_pool(name="stGt", bufs=2) as stGt:
            rb4 = stG.tile([128, HT], F32)
            nb4 = stG.tile([128, HT], F32)
            emit_stats(h3_sb[:], 4, HT, [(0, HT)], rb4, nb4, 0, stGt,
                       _r(ones_c[:]))
            emit_ffn(h3_bf[:], h3_sb[:], rb4, nb4, h4_sb[:], None,
                     wg1_d, sg1_d, bg1_d, wg2_d, bg2_d, "f2")

        # ================= Stage H: LN5 + output =================
        with tc.tile_pool(name="stH", bufs=1) as stH, \
             tc.tile_pool(name="stHt", bufs=2) as stHt:
            g5_sb = stH.tile([128, 4], F32)
            nc.sync.dma_start(out=g5_sb[:], in_=g5_d[:])
            b5_sb = stH.tile([128, 4], F32)
            nc.sync.dma_start(out=b5_sb[:], in_=b5_d[:])
            rb5 = stH.tile([128, HT], F32)
            nb5 = stH.tile([128, HT], F32)
            emit_stats(h4_sb[:], 4, HT, [(0, HT)], rb5, nb5, 0, stHt,
                       _r(ones_c[:]))
            for c in range(4):
                u = stHt.tile([128, HT], F32, tag="h_u")
                nc.vector.tensor_mul(u[:], h4_sb[:, c, :], rb5[:])
                w_ = stHt.tile([128, HT], F32, tag="h_w")
                nc.vector.tensor_add(w_[:], u[:], nb5[:])
                xn5 = stHt.tile([128, HT], F32, tag="h_o")
                nc.vector.tensor_scalar(xn5[:], w_[:],
                                        g5_sb[:, c:c + 1], b5_sb[:, c:c + 1],
                                        ALU.mult, ALU.add)
                nc.sync.dma_start(out=out_d[c], in_=xn5[:])
    return nc


_CACHE = {}


def build_nc():
    if "nc" not in _CACHE:
        nc = bacc.Bacc("TRN2", target_bir_lowering=False, debug=False,
                       num_devices=NCORES)
        _emit(nc)
        nc.compile()
        _CACHE["nc"] = nc
    return _CACHE["nc"]


def _chunk_cf(a2d):
    """[Dany, W] -> [Dany//128, 128, W] chunk-major channels-first."""
    d, w = a2d.shape
    return np.ascontiguousarray(a2d.reshape(d // 128, 128, w), dtype=np.float32)


def to_bf16(a):
    import ml_dtypes
    return np.asarray(a, dtype=np.float32).astype(ml_dtypes.bfloat16)


def round_bf16(a):
    import ml_dtypes
    return np.asarray(a, dtype=np.float32).astype(
        ml_dtypes.bfloat16).astype(np.float32)


def _pcol(vec):
    """[Dout] per-channel vector -> [128, Dout//128] (partition, chunk)."""
    n = vec.shape[0]
    return np.ascontiguousarray(vec.reshape(n // 128, 128).T, dtype=np.float32)


def make_in_maps(inputs):
    inputs = {k: np.asarray(v, dtype=np.float32) for k, v in inputs.items()}
    x = inputs["x"]; pos_emb = inputs["pos_emb"]
    ln1_g, ln1_b = inputs["ln1_g"], inputs["ln1_b"]
    ln2_g, ln2_b = inputs["ln2_g"], inputs["ln2_b"]
    ln3_g, ln3_b = inputs["ln3_g"], inputs["ln3_b"]
    ln4_g, ln4_b = inputs["ln4_g"], inputs["ln4_b"]
    ln5_g, ln5_b = inputs["ln5_g"], inputs["ln5_b"]

    # FFN1/FFN2: W' = diag(g) W (bf16), b' = b + ln_b @ W, S = colsum(W')
    w1f = round_bf16(ln1_g[:, None] * inputs["ff1_w1"])
    b1f = inputs["ff1_b1"] + ln1_b @ inputs["ff1_w1"]
    s1f = w1f.sum(axis=0)
    w2f = round_bf16(0.5 * inputs["ff1_w2"]); b2f = 0.5 * inputs["ff1_b2"]
    wg1f = round_bf16(ln4_g[:, None] * inputs["ff2_w1"])
    bg1f = inputs["ff2_b1"] + ln4_b @ inputs["ff2_w1"]
    sg1f = wg1f.sum(axis=0)
    wg2f = round_bf16(0.5 * inputs["ff2_w2"]); bg2f = 0.5 * inputs["ff2_b2"]

    s = DK ** -0.5
    pos_u_f = inputs["pos_u"].reshape(D); pos_v_f = inputs["pos_v"].reshape(D)
    wqf = round_bf16(s * (ln2_g[:, None] * inputs["wq"]))
    bqf = s * (inputs["bq"] + ln2_b @ inputs["wq"] + pos_u_f)
    sqf = wqf.sum(axis=0)
    dqvf = s * (pos_v_f - pos_u_f)
    wkf = round_bf16(ln2_g[:, None] * inputs["wk"])
    bkf = inputs["bk"] + ln2_b @ inputs["wk"]
    skf = wkf.sum(axis=0)
    wvf = round_bf16(ln2_g[:, None] * inputs["wv"])
    bvf = inputs["bv"] + ln2_b @ inputs["wv"]
    svf = wvf.sum(axis=0)
    posT = np.zeros((D, 2048), dtype=np.float32)
    posT[:, :PB] = pos_emb[0].T

    pw1f = round_bf16((inputs["pw1_w"] * ln3_g[None, :]).T)    # [512, 1024]
    bpw1f = inputs["pw1_b"] + inputs["pw1_w"] @ ln3_b          # [1024]
    spwf = pw1f.sum(axis=0)
    dwwf = inputs["dw_w"][:, 0, :]                             # [512, 31]
    pw2f = round_bf16(inputs["pw2_w"].T)                       # [512, 512]

    base = {
        "wf1": to_bf16(_chunk_cf(w1f)), "sf1": _pcol(s1f), "bf1": _pcol(b1f),
        "wf2": to_bf16(_chunk_cf(w2f)), "bf2": _pcol(b2f),
        "wg1": to_bf16(_chunk_cf(wg1f)), "sg1": _pcol(sg1f),
        "bg1": _pcol(bg1f),
        "wg2": to_bf16(_chunk_cf(wg2f)), "bg2": _pcol(bg2f),
        "posT": to_bf16(_chunk_cf(posT)),
        "pw1": to_bf16(_chunk_cf(pw1f)), "spw": _pcol(spwf),
        "bpw1": _pcol(bpw1f),
        "dwv": np.ascontiguousarray(
            dwwf.reshape(4, 128, KCONV).transpose(1, 0, 2),
            dtype=np.float32),
        "bng": _pcol(inputs["bn_g"]), "bnb": _pcol(inputs["bn_b"]),
        "pw2": to_bf16(_chunk_cf(pw2f)), "bpw2": _pcol(inputs["pw2_b"]),
        "bo": _pcol(inputs["bo"]),
        "g5": _pcol(ln5_g), "b5": _pcol(ln5_b),
        "onc": np.ones((128, 1), dtype=np.float32),
        "onc_bf": to_bf16(np.ones((128, 1))),
        "idn": np.eye(128, dtype=np.float32),
    }

    in_maps = []
    for c in range(NCORES):
        b, scr = c // 2, c % 2
        cols = slice(256 * scr, 256 * scr + 256)
        m = dict(base)
        xb = x[b, scr * HT:(scr + 1) * HT, :].T               # [512, 512]
        m["x_loc"] = _chunk_cf(xb)
        m["x_bf"] = to_bf16(_chunk_cf(xb))
        m["wq"] = to_bf16(_chunk_cf(wqf[:, cols]))
        m["sq"] = _pcol(sqf[cols]); m["bq"] = _pcol(bqf[cols])
        m["dqv"] = _pcol(bqf[cols] + dqvf[cols])   # bqv = bq + dqv
        m["wk"] = to_bf16(_chunk_cf(wkf[:, cols]))
        m["sk"] = _pcol(skf[cols]); m["bk"] = _pcol(bkf[cols])
        m["wv"] = to_bf16(_chunk_cf(wvf[:, cols]))
        m["svrow"] = np.ascontiguousarray(svf[cols].reshape(1, 256),
                                          dtype=np.float32)
        m["bvrow"] = np.ascontiguousarray(bvf[cols].reshape(1, 256),
                                          dtype=np.float32)
        m["wp"] = to_bf16(_chunk_cf(inputs["wp"][:, cols]))
        wo_rows = inputs["wo"][cols, :]                       # [256, 512]
        m["wo"] = to_bf16(np.ascontiguousarray(wo_rows.reshape(4, 64, D)))
        cmask = np.ones((1, WIN), dtype=np.float32)
        if scr == 0:
            cmask[0, :16] = 0.0
        else:
            cmask[0, WIN - 16:] = 0.0
        m["cmask"] = cmask
        in_maps.append(m)
    return in_maps


def assemble_out(results):
    out = np.empty((B, T, D), dtype=np.float32)
    for c in range(NCORES):
        b, scr = c // 2, c % 2
        ol = np.asarray(results[c]["out_loc"])                # [4, 128, 512]
        out[b, scr * HT:(scr + 1) * HT, :] = ol.reshape(D, HT).T
    return out


def kernel(**inputs):
    in_maps = make_in_maps(inputs)
    nc = build_nc()
    res = run_bass_kernel_spmd(nc, in_maps, list(range(NCORES)))
    return assemble_out(res.results)


# revision 33
# speedup vs baseline: 1.0759x; 1.0759x over previous
"""Conformer layer on 8 Trainium2 NeuronCores (v2).

Sharding: core c handles batch b=c//2. Within a batch pair:
 - token-parallel (halves of T=1024) for FFN1/conv/FFN2/LN stages,
 - head-parallel (4 heads each) for attention.

v2 changes vs baseline:
 - LayerNorm fold: matmuls consume raw activations; per-token scale/shift is
   applied as a post-matmul fixup z = psy*rec + colsum(W')*(-m*rec) + b', so
   the PE never waits for LN statistics (keeps the HAM clock warm).
 - bf16 operands for attention (q/k/v/p/probs/o_h) with moving-dim-1024
   matmuls, bf16 FFN/pw/wo weights (same PE rate, half the DMA).
 - AllGather carries bf16 h and overlaps with p-projection + local-half QKV;
   a warmup collective at kernel start absorbs the first-cc latency.
 - Attention ReduceScatter in bf16.
 - Depthwise-conv diagonal matrices built on-chip (saves an 8MB DMA).
 - Plain loads on HWDGE (nc.sync), freeing GpSimd for casts/broadcasts.
"""

import numpy as np

import concourse.bass as bass
import concourse.mybir as mybir
import concourse.tile as tile
from concourse import bacc
from concourse.bass import ds, ts
from concourse.bass_utils import run_bass_kernel_spmd
from contextlib import ExitStack

F32 = mybir.dt.float32
F32R = mybir.dt.float32r
BF16 = mybir.dt.bfloat16
AF = mybir.ActivationFunctionType
ALU = mybir.AluOpType

D, DFF, H, DK, KCONV = 512, 2048, 8, 64, 31
B, T = 4, 1024
EPS = 1e-5
HT = 512            # tokens per core
WIN = 544           # conv window: 16 + 512 + 16
PB = 2047
BAND = 1152         # bd band width per q-chunk
BST = 1160          # bd dram row stride (elements)
NCORES = 8

PAIRS = [[0, 1], [2, 3], [4, 5], [6, 7]]
ALLG = [[0, 1, 2, 3, 4, 5, 6, 7]]


def _r(ap):
    return ap.bitcast(F32R)


def _emit(nc):
    def inp(name, shape, dt=F32):
        return nc.dram_tensor(name, list(shape), dt, kind="ExternalInput")

    x_d = inp("x_loc", (4, 128, HT), F32R)
    xbf_d = inp("x_bf", (4, 128, HT), BF16)
    wf1_d = inp("wf1", (4, 128, DFF), BF16)
    sf1_d = inp("sf1", (128, 16)); bf1_d = inp("bf1", (128, 16))
    wf2_d = inp("wf2", (16, 128, D), BF16); bf2_d = inp("bf2", (128, 4))
    wq_d = inp("wq", (4, 128, 256), BF16)
    sq_d = inp("sq", (128, 2)); bq_d = inp("bq", (128, 2))
    dqv_d = inp("dqv", (128, 2))
    wk_d = inp("wk", (4, 128, 256), BF16)
    sk_d = inp("sk", (128, 2)); bk_d = inp("bk", (128, 2))
    wv_d = inp("wv", (4, 128, 256), BF16)
    sv_d = inp("svrow", (1, 256)); bv_d = inp("bvrow", (1, 256))
    wp_d = inp("wp", (4, 128, 256), BF16)
    wo_d = inp("wo", (4, 64, D), BF16); bo_d = inp("bo", (128, 4))
    posT_d = inp("posT", (4, 128, 2048), BF16)
    pw1_d = inp("pw1", (4, 128, 1024), BF16)
    spw_d = inp("spw", (128, 8)); bpw1_d = inp("bpw1", (128, 8))
    dwv_d = inp("dwv", (128, 4, KCONV))
    bng_d = inp("bng", (128, 4)); bnb_d = inp("bnb", (128, 4))
    pw2_d = inp("pw2", (4, 128, D), BF16); bpw2_d = inp("bpw2", (128, 4))
    cmask_d = inp("cmask", (1, WIN))
    wg1_d = inp("wg1", (4, 128, DFF), BF16)
    sg1_d = inp("sg1", (128, 16)); bg1_d = inp("bg1", (128, 16))
    wg2_d = inp("wg2", (16, 128, D), BF16); bg2_d = inp("bg2", (128, 4))
    g5_d = inp("g5", (128, 4)); b5_d = inp("b5", (128, 4))
    onc_d = inp("onc", (128, 1), F32R)
    oncb_d = inp("onc_bf", (128, 1), BF16)
    idn_d = inp("idn", (128, 128))

    out_d = nc.dram_tensor("out_loc", [4, 128, HT], F32, kind="ExternalOutput")

    cc_w_in = nc.dram_tensor("cc_w_in", [1, 64], F32)
    cc_w_out = nc.dram_tensor("cc_w_out", [2, 64], F32)
    cc_h_in = nc.dram_tensor("cc_h_in", [4, 128, HT], BF16)
    cc_h_out = nc.dram_tensor("cc_h_out", [2, 4, 128, HT], BF16)
    cc_a_in = nc.dram_tensor("cc_a_in", [2, 4, 128, WIN], BF16)
    cc_a_out = nc.dram_tensor("cc_a_out", [4, 128, WIN], BF16)
    cc_bn_in = nc.dram_tensor("cc_bn_in", [128, 8], F32)
    cc_bn_out = nc.dram_tensor("cc_bn_out", [128, 8], F32)
    bd_d = [nc.dram_tensor(f"bd_{i}", [128 * BST], BF16) for i in range(32)]

    uid = [0]

    with tile.TileContext(nc) as tc, ExitStack() as ctx:
        const = ctx.enter_context(tc.tile_pool(name="const", bufs=1))
        ones_c = const.tile([128, 1], F32R)
        nc.sync.dma_start(out=ones_c[:], in_=onc_d[:])
        ones_cb = const.tile([128, 1], BF16)
        nc.sync.dma_start(out=ones_cb[:], in_=oncb_d[:])
        eps1 = const.tile([1, 1], F32); nc.vector.memset(eps1[:], EPS)
        epsP = const.tile([128, 1], F32); nc.vector.memset(epsP[:], EPS)
        identb = const.tile([128, 128], BF16)
        nc.gpsimd.dma_start(out=identb[:], in_=idn_d[:])

        # CC-stream warmup: tiny 8-way AllReduce absorbs the bootstrap
        # barrier + cross-core startup skew while FFN1 runs.
        warm = const.tile([1, 64], F32)
        nc.vector.memset(warm[:], 0.0)
        nc.gpsimd.dma_start(out=cc_w_in[:], in_=warm[:])
        nc.gpsimd.collective_compute(
            "AllReduce", ALU.add, ins=[cc_w_in[:]], outs=[cc_w_out[0:1]],
            replica_groups=ALLG)

        # per-engine copies of the core's token-half index (register values
        # are engine-local)
        pidv = nc.vector.partition_id()
        scv, scv2 = pidv % 2, (pidv + 1) % 2
        pida = nc.scalar.partition_id()
        sca, sca2 = pida % 2, (pida + 1) % 2
        pids = nc.sync.partition_id()
        scs, scs2 = pids % 2, (pids + 1) % 2

        act = ctx.enter_context(tc.tile_pool(name="act", bufs=1))
        hfe_sb = act.tile([128, 4, WIN], F32R)   # conv residual window
        h_sb = act.tile([128, 4, HT], F32R)      # post-FFN1 hidden (local)
        h_bf = act.tile([128, 4, HT], BF16)
        h3_sb = act.tile([128, 4, HT], F32R)     # post-conv hidden
        h3_bf = act.tile([128, 4, HT], BF16)
        h4_sb = act.tile([128, 4, HT], F32R)     # post-FFN2 hidden

        # ---------- LN statistics (fold form) ----------
        # Produces broadcast tiles RB = 1/std and NM = -mean/std per token.
        def emit_stats(x4, nchunk, W, blocks, rb_t, nb_t, col0, sbp, ones):
            uid[0] += 1
            with tc.tile_pool(name=f"lnps{uid[0]}", bufs=1,
                              space="PSUM") as lnps:
                x2 = sbp.tile([128, nchunk, W], F32R, tag="ln_sq")
                nc.vector.tensor_mul(x2[:], x4, x4)
                for b0, bw in blocks:
                    pss = lnps.tile([1, bw], F32, tag="lns")
                    psq = lnps.tile([1, bw], F32, tag="lnq")
                    for c in range(nchunk):
                        nc.tensor.matmul(pss[:], ones,
                                         x4[:, c, b0:b0 + bw],
                                         start=(c == 0), stop=(c == nchunk - 1))
                    for c in range(nchunk):
                        nc.tensor.matmul(psq[:], _r(ones_c[:]),
                                         _r(x2[:, c, b0:b0 + bw]),
                                         start=(c == 0), stop=(c == nchunk - 1))
                    mng = sbp.tile([1, bw], F32, tag="ln_m")
                    nc.scalar.activation(mng[:], pss[:], AF.Copy,
                                         scale=-1.0 / D)
                    e2 = sbp.tile([1, bw], F32, tag="ln_e2")
                    nc.scalar.activation(e2[:], psq[:], AF.Copy, scale=1.0 / D)
                    var = sbp.tile([1, bw], F32, tag="ln_var")
                    nc.vector.tensor_mul(var[:], mng[:], mng[:])
                    nc.vector.tensor_sub(var[:], e2[:], var[:])
                    sd = sbp.tile([1, bw], F32, tag="ln_sd")
                    nc.scalar.activation(sd[:], var[:], AF.Sqrt, bias=eps1[:])
                    rec = sbp.tile([1, bw], F32, tag="ln_rs")
                    scr = sbp.tile([1, bw], F32, tag="ln_scr")
                    nc.vector.reciprocal_approx_accurate(rec[:], sd[:], scr[:])
                    nmr = sbp.tile([1, bw], F32, tag="ln_nm")
                    nc.vector.tensor_mul(nmr[:], mng[:], rec[:])
                    nc.gpsimd.partition_broadcast(
                        rb_t[:, col0 + b0:col0 + b0 + bw], rec[:])
                    nc.gpsimd.partition_broadcast(
                        nb_t[:, col0 + b0:col0 + b0 + bw], nmr[:])

        # ---------- FFN with LN fold ----------
        def emit_ffn(xbf, xres, rb_t, nb_t, out, outbf, w1dram, s1dram,
                     b1dram, w2dram, b2dram, pref):
            with tc.tile_pool(name=pref + "w", bufs=1) as wp_, \
                 tc.tile_pool(name=pref + "t", bufs=3) as tp, \
                 tc.tile_pool(name=pref + "ps", bufs=2, space="PSUM") as psp, \
                 tc.tile_pool(name=pref + "ph", bufs=1, space="PSUM") as php:
                w1 = wp_.tile([128, 4, DFF], BF16)
                for c in range(4):
                    nc.sync.dma_start(out=w1[:, c, :], in_=w1dram[c])
                w2 = wp_.tile([128, 16, D], BF16)
                for j in range(16):
                    nc.sync.dma_start(out=w2[:, j, :], in_=w2dram[j])
                s1 = wp_.tile([128, 16], F32)
                nc.sync.dma_start(out=s1[:], in_=s1dram[:])
                b1 = wp_.tile([128, 16], F32)
                nc.sync.dma_start(out=b1[:], in_=b1dram[:])
                b2 = wp_.tile([128, 4], F32)
                nc.sync.dma_start(out=b2[:], in_=b2dram[:])
                psh = php.tile([128, 4, HT], F32)
                for j in range(16):
                    psy = psp.tile([128, HT], F32, tag="psy")
                    for c in range(4):
                        nc.tensor.matmul(psy[:], w1[:, c, ts(j, 128)],
                                         xbf[:, c, :],
                                         start=(c == 0), stop=(c == 3))
                    u = tp.tile([128, HT], F32, tag="u")
                    nc.vector.tensor_mul(u[:], psy[:], rb_t[:])
                    z = tp.tile([128, HT], F32, tag="z")
                    nc.vector.scalar_tensor_tensor(
                        out=z[:], in0=nb_t[:], scalar=s1[:, j:j + 1],
                        in1=u[:], op0=ALU.mult, op1=ALU.add)
                    sig = tp.tile([128, HT], F32, tag="sig")
                    nc.scalar.activation(sig[:], z[:], AF.Sigmoid,
                                         bias=b1[:, j:j + 1])
                    y1 = tp.tile([128, HT], BF16, tag="y1")
                    nc.vector.scalar_tensor_tensor(
                        out=y1[:], in0=z[:], scalar=b1[:, j:j + 1],
                        in1=sig[:], op0=ALU.add, op1=ALU.mult)
                    for f in range(4):
                        nc.tensor.matmul(psh[:, f, :],
                                         w2[:, j, ts(f, 128)], y1[:],
                                         start=(j == 0), stop=(j == 15))
                for c in range(4):
                    nc.vector.scalar_tensor_tensor(
                        out=out[:, c, :], in0=psh[:, c, :],
                        scalar=b2[:, c:c + 1], in1=xres[:, c, :],
                        op0=ALU.add, op1=ALU.add)
                    if outbf is not None:
                        nc.vector.tensor_copy(outbf[:, c, :],
                                              out[:, c, :].bitcast(F32))

        # ================= Stage A: FFN1 on local tokens =================
        with tc.tile_pool(name="stA", bufs=1) as stA, \
             tc.tile_pool(name="stAt", bufs=2) as stAt:
            x_sb = stA.tile([128, 4, HT], F32R)
            for c in range(4):
                nc.gpsimd.dma_start(out=x_sb[:, c, :], in_=x_d[c])
            x_bf = stA.tile([128, 4, HT], BF16)
            for c in range(4):
                nc.sync.dma_start(out=x_bf[:, c, :], in_=xbf_d[c])
            rb1 = stA.tile([128, HT], F32)
            nb1 = stA.tile([128, HT], F32)
            emit_stats(x_sb[:], 4, HT, [(0, HT)], rb1, nb1, 0, stAt,
                       _r(ones_c[:]))
            emit_ffn(x_bf[:], x_sb[:], rb1, nb1, h_sb[:], h_bf[:],
                     wf1_d, sf1_d, bf1_d, wf2_d, bf2_d, "f1")
            # conv residual window: local half occupies [16, 528)
            for c in range(4):
                nc.vector.tensor_copy(hfe_sb[:, c, 16:16 + HT].bitcast(F32),
                                      h_sb[:, c, :].bitcast(F32))
            for c in range(4):
                nc.sync.dma_start(out=cc_h_in[c], in_=h_bf[:, c, :])

        nc.gpsimd.collective_compute(
            "AllGather", ALU.bypass, ins=[cc_h_in[:]], outs=[cc_h_out[:]],
            replica_groups=PAIRS)

        # ============ attention scope ============
        with tc.tile_pool(name="attp", bufs=1) as attp:
            woff = sca * HT                # local half start (ACT offsets)
            roff = sca2 * HT               # remote half start

            q_sb = attp.tile([128, 2, T], BF16)
            qv_sb = attp.tile([128, 2, T], BF16)
            k_sb = attp.tile([128, 2, T], BF16)
            v65 = attp.tile([128, 8, 4, 65], BF16)
            nc.vector.memset(v65[:, :, :, 64:65], 1.0)
            p_sb = attp.tile([128, 2, 2048], BF16)
            # rb2/nb2 and recc/nmc are stored local-half-first (static
            # writes); the eviction target offsets stay symbolic.
            rb2 = attp.tile([128, T], F32)
            nb2 = attp.tile([128, T], F32)
            recc = attp.tile([128, 8], F32)
            nmc = attp.tile([128, 8], F32)

            # ---- p-projection (overlaps AllGather) ----
            with tc.tile_pool(name="ppp", bufs=2, space="PSUM") as ppp, \
                 tc.tile_pool(name="atw", bufs=1) as atw:
                wp_sb = atw.tile([128, 4, 256], BF16, tag="wp")
                for c in range(4):
                    nc.sync.dma_start(out=wp_sb[:, c, :], in_=wp_d[c])
                pos_sb = atw.tile([128, 4, 2048], BF16, tag="pos")
                for c in range(4):
                    nc.sync.dma_start(out=pos_sb[:, c, :], in_=posT_d[c])
                for m in range(2):
                    for pc in range(4):
                        psp_t = ppp.tile([128, 512], F32, tag="psp")
                        for c in range(4):
                            nc.tensor.matmul(
                                psp_t[:], wp_sb[:, c, ts(m, 128)],
                                pos_sb[:, c, ts(pc, 512)],
                                start=(c == 0), stop=(c == 3))
                        nc.scalar.activation(p_sb[:, m, ts(pc, 512)],
                                             psp_t[:], AF.Copy)

            # ---- local-half LN2 stats + qkv (overlap AllGather) ----
            with tc.tile_pool(name="stC", bufs=1) as stC, \
                 tc.tile_pool(name="stCt", bufs=2) as stCt, \
                 tc.tile_pool(name="qkp", bufs=2, space="PSUM") as qkps:

                def emit_stats2(x4, ones, rcol):
                    # LN2 stats for one half; writes rb2/nb2[:, rcol:rcol+HT]
                    # and recc/nmc[:, rcol//128 : +4] (static columns).
                    uid[0] += 1
                    cc = rcol // 128
                    with tc.tile_pool(name=f"lnps{uid[0]}", bufs=1,
                                      space="PSUM") as lnps:
                        x2 = stCt.tile([128, 4, HT], F32R, tag="ln_sq")
                        nc.vector.tensor_mul(x2[:], x4, x4)
                        pss = lnps.tile([1, HT], F32, tag="lns")
                        psq = lnps.tile([1, HT], F32, tag="lnq")
                        for c in range(4):
                            nc.tensor.matmul(pss[:], ones, x4[:, c, :],
                                             start=(c == 0), stop=(c == 3))
                        for c in range(4):
                            nc.tensor.matmul(psq[:], _r(ones_c[:]),
                                             _r(x2[:, c, :]),
                                             start=(c == 0), stop=(c == 3))
                        mng = stCt.tile([1, HT], F32, tag="ln_m")
                        nc.scalar.activation(mng[:], pss[:], AF.Copy,
                                             scale=-1.0 / D)
                        e2 = stCt.tile([1, HT], F32, tag="ln_e2")
                        nc.scalar.activation(e2[:], psq[:], AF.Copy,
                                             scale=1.0 / D)
                        var = stCt.tile([1, HT], F32, tag="ln_var")
                        nc.vector.tensor_mul(var[:], mng[:], mng[:])
                        nc.vector.tensor_sub(var[:], e2[:], var[:])
                        sd = stCt.tile([1, HT], F32, tag="ln_sd")
                        nc.scalar.activation(sd[:], var[:], AF.Sqrt,
                                             bias=eps1[:])
                        rec2 = stCt.tile([1, HT], F32, tag="ln_rec")
                        scr = stCt.tile([1, HT], F32, tag="ln_scr")
                        nc.vector.reciprocal_approx_accurate(rec2[:], sd[:],
                                                             scr[:])
                        nm2 = stCt.tile([1, HT], F32, tag="ln_nm")
                        nc.vector.tensor_mul(nm2[:], mng[:], rec2[:])
                        nc.gpsimd.partition_broadcast(
                            rb2[:, rcol:rcol + HT], rec2[:])
                        nc.gpsimd.partition_broadcast(
                            nb2[:, rcol:rcol + HT], nm2[:])
                        for b in range(4):
                            nc.gpsimd.dma_start(
                                out=recc[:, cc + b:cc + b + 1],
                                in_=rec2[:, ts(b, 128)])
                            nc.gpsimd.dma_start(
                                out=nmc[:, cc + b:cc + b + 1],
                                in_=nm2[:, ts(b, 128)])

                wq_sb = stC.tile([128, 4, 256], BF16)
                wk_sb = stC.tile([128, 4, 256], BF16)
                wv_sb = stC.tile([128, 4, 256], BF16)
                for c in range(4):
                    nc.sync.dma_start(out=wq_sb[:, c, :], in_=wq_d[c])
                    nc.sync.dma_start(out=wk_sb[:, c, :], in_=wk_d[c])
                    nc.sync.dma_start(out=wv_sb[:, c, :], in_=wv_d[c])
                sq_sb = stC.tile([128, 2], F32)
                nc.sync.dma_start(out=sq_sb[:], in_=sq_d[:])
                bq_sb = stC.tile([128, 2], F32)
                nc.sync.dma_start(out=bq_sb[:], in_=bq_d[:])
                sk_sb = stC.tile([128, 2], F32)
                nc.sync.dma_start(out=sk_sb[:], in_=sk_d[:])
                bk_sb = stC.tile([128, 2], F32)
                nc.sync.dma_start(out=bk_sb[:], in_=bk_d[:])
                bqv_sb = stC.tile([128, 2], F32)
                nc.sync.dma_start(out=bqv_sb[:], in_=dqv_d[:])
                sv_row = stC.tile([1, 256], F32)
                nc.sync.dma_start(out=sv_row[:], in_=sv_d[:])
                svb = stC.tile([128, 256], F32)
                nc.gpsimd.partition_broadcast(svb[:], sv_row[:])
                bv_row = stC.tile([1, 256], F32)
                nc.sync.dma_start(out=bv_row[:], in_=bv_d[:])
                bvb = stC.tile([128, 256], F32)
                nc.gpsimd.partition_broadcast(bvb[:], bv_row[:])

                def emit_qk(xin, rcol, off):
                    # rcol: static column base in rb2/nb2 (local-first);
                    # off: symbolic global token offset for the outputs.
                    for m in range(2):
                        for w_sb, s_w, b_w, dst in (
                                (wq_sb, sq_sb, bq_sb, q_sb),
                                (wk_sb, sk_sb, bk_sb, k_sb)):
                            ps = qkps.tile([128, HT], F32, tag="psqk")
                            for c in range(4):
                                nc.tensor.matmul(
                                    ps[:], w_sb[:, c, ts(m, 128)],
                                    xin[:, c, :],
                                    start=(c == 0), stop=(c == 3))
                            u = stCt.tile([128, HT], F32, tag="qk_u")
                            nc.vector.tensor_mul(u[:], ps[:],
                                                 rb2[:, rcol:rcol + HT])
                            w_ = stCt.tile([128, HT], F32, tag="qk_w")
                            nc.vector.scalar_tensor_tensor(
                                out=w_[:], in0=nb2[:, rcol:rcol + HT],
                                scalar=s_w[:, m:m + 1], in1=u[:],
                                op0=ALU.mult, op1=ALU.add)
                            nc.scalar.activation(
                                dst[:, m, ds(off, HT)], w_[:], AF.Identity,
                                bias=b_w[:, m:m + 1])
                            if dst is q_sb:
                                nc.scalar.activation(
                                    qv_sb[:, m, ds(off, HT)], w_[:],
                                    AF.Identity, bias=bqv_sb[:, m:m + 1])

                def emit_v(xin, vbase, rbase):
                    # vbase: symbolic global chunk base; rbase: static
                    # column base into recc/nmc.
                    for tq in range(4):
                        psv = qkps.tile([128, 256], F32, tag="psv")
                        for c in range(4):
                            nc.tensor.matmul(
                                psv[:], xin[:, c, ts(tq, 128)],
                                wv_sb[:, c, :],
                                start=(c == 0), stop=(c == 3))
                        u = stCt.tile([128, 256], F32, tag="v_u")
                        nc.vector.tensor_scalar_mul(
                            u[:], psv[:], recc[:, rbase + tq:rbase + tq + 1])
                        w_ = stCt.tile([128, 256], F32, tag="v_w")
                        nc.vector.scalar_tensor_tensor(
                            out=w_[:], in0=svb[:],
                            scalar=nmc[:, rbase + tq:rbase + tq + 1],
                            in1=u[:], op0=ALU.mult, op1=ALU.add)
                        vt = stCt.tile([128, 256], BF16, tag="v_t")
                        nc.vector.tensor_add(vt[:], w_[:], bvb[:])
                        nc.vector.tensor_copy(
                            v65[:, ds(vbase + tq, 1), :, 0:64]
                            .rearrange("p o h d -> p (o h) d"),
                            vt[:].rearrange("p (h d) -> p h d", h=4))

                emit_stats2(h_sb[:], _r(ones_c[:]), 0)
                emit_qk(h_bf[:], 0, woff)
                emit_v(h_bf[:], scv * 4, 0)

                # ---- gather lands: remote half ----
                h_rem = stC.tile([128, 4, HT], BF16)
                for c in range(4):
                    nc.sync.dma_start(
                        out=h_rem[:, c, :],
                        in_=cc_h_out[:].rearrange("s c p t -> (s c) p t")
                        [ds(scs2 * 4 + c, 1)]
                        .rearrange("o p t -> (o p) t"))
                # conv halo: zero edges then copy 16 remote tokens
                nc.vector.memset(hfe_sb[:, :, 0:16].bitcast(F32), 0.0)
                nc.vector.memset(hfe_sb[:, :, 16 + HT:].bitcast(F32), 0.0)
                for c in range(4):
                    nc.vector.tensor_copy(
                        hfe_sb[:, c, ds(scv2 * (16 + HT), 16)].bitcast(F32),
                        h_rem[:, c, ds(scv * (HT - 16), 16)])

                emit_stats2(h_rem[:], ones_cb[:], HT)
                emit_qk(h_rem[:], HT, roff)
                emit_v(h_rem[:], scv2 * 4, 4)

            # ---- bd banded matmuls for all heads ----
            with tc.tile_pool(name="bds", bufs=2) as bdp, \
                 tc.tile_pool(name="psb", bufs=2, space="PSUM") as psb:
                for h in range(4):
                    hc, hr = h // 2, 64 * (h % 2)
                    for qc in range(8):
                        base = max(0, 895 - 128 * qc)
                        bdw = bdp.tile([128, BAND], BF16, tag="bdw")
                        for pi in range(3):
                            psB = psb.tile([128, 384], F32, tag="psB")
                            nc.tensor.matmul(
                                psB[:],
                                qv_sb[hr:hr + 64, hc, ts(qc, 128)],
                                p_sb[hr:hr + 64, hc,
                                     base + pi * 384: base + (pi + 1) * 384],
                                start=True, stop=True)
                            nc.vector.tensor_copy(
                                bdw[:, pi * 384:(pi + 1) * 384], psB[:])
                        nc.sync.dma_start(
                            out=bd_d[h * 8 + qc][:].rearrange(
                                "(p w) -> p w", p=128)[:, 0:BAND],
                            in_=bdw[:])

            # ---- attention heads: scores + softmax + AV ----
            o_h = [attp.tile([64, T], BF16, name=f"o_h{_h}", tag=f"o_h{_h}")
                   for _h in range(4)]
            with tc.tile_pool(name="bdsh", bufs=1) as shp, \
                 tc.tile_pool(name="atp", bufs=3) as atp, \
                 tc.tile_pool(name="atn", bufs=1) as atn, \
                 tc.tile_pool(name="pss", bufs=2, space="PSUM") as pss_p, \
                 tc.tile_pool(name="psav", bufs=2, space="PSUM") as psav:
                for h in range(4):
                    hc, hr = h // 2, 64 * (h % 2)
                    # XBAR transpose during the shifted read:
                    # bdshT[p, kc, q] = bd[q, kc*128+p]  (k-major layout)
                    bdshT = shp.tile([128, 8, T], BF16, tag=f"sh{h % 2}")
                    for qc in range(8):
                        c0 = 127 if qc == 7 else 128
                        src = bass.AP(tensor=bd_d[h * 8 + qc], offset=c0,
                                      ap=[[BST - 1, 128], [1, T]])
                        nc.scalar.dma_start(
                            out=bdshT[:, :, ts(qc, 128)], in_=src,
                            transpose=True)
                    psA = psav.tile([65, T], F32, tag="psAV")
                    for kc in range(8):
                        psS = pss_p.tile([128, T], F32, tag="psS")
                        for th in range(2):
                            nc.tensor.matmul(
                                psS[:, th * 512:(th + 1) * 512],
                                k_sb[hr:hr + 64, hc, ts(kc, 128)],
                                q_sb[hr:hr + 64, hc,
                                     th * 512:(th + 1) * 512],
                                start=True, stop=True, skip_group_check=True)
                        zsc = atp.tile([128, T], F32, tag="zsc")
                        nc.vector.tensor_add(zsc[:], psS[:], bdshT[:, kc, :])
                        probs = atp.tile([128, T], BF16, tag="probs")
                        nc.scalar.activation(probs[:], zsc[:], AF.Exp)
                        for th in range(2):
                            nc.tensor.matmul(
                                psA[:, th * 512:(th + 1) * 512],
                                v65[:, kc, h, :],
                                probs[:, th * 512:(th + 1) * 512],
                                start=(kc == 0), stop=(kc == 7),
                                skip_group_check=True)
                    s65 = atn.tile([65, T], F32, tag="s65")
                    nc.vector.tensor_copy(s65[64:65, :], psA[64:65, :])
                    row = atn.tile([1, T], F32, tag="row")
                    nc.gpsimd.dma_start(out=row[:], in_=s65[64:65, :])
                    rec = atn.tile([1, T], F32, tag="rec")
                    scr = atn.tile([1, T], F32, tag="scr")
                    nc.vector.reciprocal_approx_accurate(rec[:], row[:],
                                                         scr[:])
                    rb = atn.tile([64, T], F32, tag="rb")
                    nc.gpsimd.partition_broadcast(rb[:], rec[:])
                    nc.vector.tensor_mul(o_h[h][:], psA[0:64, :], rb[:])

            # ---- out-projection + pair ReduceScatter (bf16) ----
            with tc.tile_pool(name="pso", bufs=2, space="PSUM") as psop, \
                 tc.tile_pool(name="aot", bufs=2) as aot:
                wo_sb = aot.tile([64, 4, D], BF16, tag="wo", bufs=1)
                for hh in range(4):
                    nc.sync.dma_start(out=wo_sb[:, hh, :], in_=wo_d[hh])
                for f in range(4):
                    pso = psop.tile([128, T], F32, tag="pso")
                    for th in range(2):
                        for hh in range(4):
                            nc.tensor.matmul(
                                pso[:, th * 512:(th + 1) * 512],
                                wo_sb[:, hh, ts(f, 128)],
                                o_h[hh][:, th * 512:(th + 1) * 512],
                                start=(hh == 0), stop=(hh == 3),
                                skip_group_check=True)
                    ao = aot.tile([128, 2, WIN], BF16, tag="ao")
                    nc.vector.memset(ao[:, 0, 0:16], 0.0)
                    nc.vector.memset(ao[:, 1, WIN - 16:WIN], 0.0)
                    nc.vector.tensor_copy(ao[:, 0, 16:WIN], pso[:, 0:528])
                    nc.vector.tensor_copy(ao[:, 1, 0:528], pso[:, 496:T])
                    nc.sync.dma_start(out=cc_a_in[0, f], in_=ao[:, 0, :])
                    nc.sync.dma_start(out=cc_a_in[1, f], in_=ao[:, 1, :])

        nc.gpsimd.collective_compute(
            "ReduceScatter", ALU.add, ins=[cc_a_in[:]], outs=[cc_a_out[:]],
            replica_groups=PAIRS)

        # ================= Stage F: conv module =================
        with tc.tile_pool(name="stF", bufs=1) as stF, \
             tc.tile_pool(name="stFt", bufs=2) as stFt:
            # on-chip depthwise diag build (overlaps ReduceScatter)
            dwv = stF.tile([128, 4, KCONV], F32)
            nc.sync.dma_start(out=dwv[:], in_=dwv_d[:])
            dg = stF.tile([128, 4, KCONV, 128], BF16)
            for c in range(4):
                for j in range(KCONV):
                    nc.vector.tensor_scalar(
                        dg[:, c, j, :], identb[:],
                        dwv[:, c, j:j + 1], 0.0, ALU.mult, ALU.add)
            w1 = stF.tile([128, 4, 1024], BF16)
            for c in range(4):
                nc.sync.dma_start(out=w1[:, c, :], in_=pw1_d[c])
            spw = stF.tile([128, 8], F32)
            nc.sync.dma_start(out=spw[:], in_=spw_d[:])
            bp1 = stF.tile([128, 8], F32)
            nc.sync.dma_start(out=bp1[:], in_=bpw1_d[:])
            w2 = stF.tile([128, 4, D], BF16)
            for c in range(4):
                nc.sync.dma_start(out=w2[:, c, :], in_=pw2_d[c])
            bp2 = stF.tile([128, 4], F32)
            nc.sync.dma_start(out=bp2[:], in_=bpw2_d[:])
            bo_sb = stF.tile([128, 4], F32)
            nc.sync.dma_start(out=bo_sb[:], in_=bo_d[:])
            cm = stF.tile([1, WIN], F32)
            nc.sync.dma_start(out=cm[:], in_=cmask_d[:])
            cmb = stF.tile([128, WIN], F32)
            nc.gpsimd.partition_broadcast(cmb[:], cm[:])

            h2w = stF.tile([128, 4, WIN], F32R)
            for c in range(4):
                at = stFt.tile([128, WIN], F32, tag="at")
                nc.gpsimd.dma_start(out=at[:], in_=cc_a_out[c])
                nc.vector.scalar_tensor_tensor(
                    out=h2w[:, c, :], in0=at[:],
                    scalar=bo_sb[:, c:c + 1],
                    in1=hfe_sb[:, c, :].bitcast(F32),
                    op0=ALU.add, op1=ALU.add)
            h2w_bf = stF.tile([128, 4, WIN], BF16)
            for c in range(4):
                nc.vector.tensor_copy(h2w_bf[:, c, :],
                                      h2w[:, c, :].bitcast(F32))

            rb3 = stF.tile([128, WIN], F32)
            nb3 = stF.tile([128, WIN], F32)
            emit_stats(h2w[:], 4, WIN, [(0, 272), (272, 272)], rb3, nb3, 0,
                       stFt, _r(ones_c[:]))

            glu = stF.tile([128, 4, WIN], BF16)
            with tc.tile_pool(name="cvp1", bufs=1, space="PSUM") as cps:
                for m in range(4):
                    psa = cps.tile([128, 2, 512], F32, tag="psa")
                    psg = cps.tile([128, 2, 512], F32, tag="psg")
                    for half in range(2):
                        sl = slice(half * 272, (half + 1) * 272)
                        for c in range(4):
                            nc.tensor.matmul(psa[:, half, 0:272],
                                             w1[:, c, ts(m, 128)],
                                             h2w_bf[:, c, sl],
                                             start=(c == 0), stop=(c == 3),
                                             skip_group_check=True)
                        for c in range(4):
                            nc.tensor.matmul(psg[:, half, 0:272],
                                             w1[:, c, 512 + m * 128:
                                                 512 + (m + 1) * 128],
                                             h2w_bf[:, c, sl],
                                             start=(c == 0), stop=(c == 3),
                                             skip_group_check=True)
                    for half in range(2):
                        sl = slice(half * 272, (half + 1) * 272)
                        ua = stFt.tile([128, 272], F32, tag="cv_ua")
                        nc.vector.tensor_mul(ua[:], psa[:, half, 0:272],
                                             rb3[:, sl])
                        za = stFt.tile([128, 272], F32, tag="cv_za")
                        nc.vector.scalar_tensor_tensor(
                            out=za[:], in0=nb3[:, sl],
                            scalar=spw[:, m:m + 1], in1=ua[:],
                            op0=ALU.mult, op1=ALU.add)
                        ug = stFt.tile([128, 272], F32, tag="cv_ug")
                        nc.vector.tensor_mul(ug[:], psg[:, half, 0:272],
                                             rb3[:, sl])
                        zg = stFt.tile([128, 272], F32, tag="cv_zg")
                        nc.vector.scalar_tensor_tensor(
                            out=zg[:], in0=nb3[:, sl],
                            scalar=spw[:, 4 + m:5 + m], in1=ug[:],
                            op0=ALU.mult, op1=ALU.add)
                        sg = stFt.tile([128, 272], F32, tag="cv_sg")
                        nc.scalar.activation(sg[:], zg[:], AF.Sigmoid,
                                             bias=bp1[:, 4 + m:5 + m])
                        gl = stFt.tile([128, 272], F32, tag="cv_gl")
                        nc.vector.scalar_tensor_tensor(
                            out=gl[:], in0=za[:], scalar=bp1[:, m:m + 1],
                            in1=sg[:], op0=ALU.add, op1=ALU.mult)
                        nc.vector.tensor_mul(glu[:, m, sl], gl[:],
                                             cmb[:, sl])

            # depthwise conv: 31 accumulating diagonal matmuls per chunk
            acc = stF.tile([128, 4, HT], F32)
            with tc.tile_pool(name="dgp", bufs=2, space="PSUM") as dgp:
                for c in range(4):
                    psC = dgp.tile([128, HT], F32, tag="psC")
                    for j in range(KCONV):
                        nc.tensor.matmul(psC[:], dg[:, c, j, :],
                                         glu[:, c, 1 + j:1 + j + HT],
                                         start=(j == 0), stop=(j == KCONV - 1))
                    nc.vector.tensor_copy(acc[:, c, :], psC[:])
            # BN stats + 8-way AllReduce
            bnpack = stF.tile([128, 8], F32)
            for c in range(4):
                bst_t = stFt.tile([128, 6], F32, tag="bst")
                nc.vector.bn_stats(bst_t[:], acc[:, c, :])
                mv = stFt.tile([128, 2], F32, tag="mv")
                nc.vector.bn_aggr(mv[:], bst_t[:])
                nc.vector.tensor_copy(bnpack[:, 2 * c:2 * c + 1], mv[:, 0:1])
                nc.vector.scalar_tensor_tensor(
                    out=bnpack[:, 2 * c + 1:2 * c + 2], in0=mv[:, 0:1],
                    scalar=mv[:, 0:1], in1=mv[:, 1:2],
                    op0=ALU.mult, op1=ALU.add)
            nc.sync.dma_start(out=cc_bn_in[:], in_=bnpack[:])
            nc.gpsimd.collective_compute(
                "AllReduce", ALU.add, ins=[cc_bn_in[:]], outs=[cc_bn_out[:]],
                replica_groups=ALLG)
            bnar = stF.tile([128, 8], F32)
            nc.sync.dma_start(out=bnar[:], in_=cc_bn_out[:])
            bng_sb = stF.tile([128, 4], F32)
            nc.sync.dma_start(out=bng_sb[:], in_=bng_d[:])
            bnb_sb = stF.tile([128, 4], F32)
            nc.sync.dma_start(out=bnb_sb[:], in_=bnb_d[:])
            ysl = stF.tile([128, 4, HT], BF16)
            for c in range(4):
                mg = stFt.tile([128, 1], F32, tag="mg")
                nc.scalar.activation(mg[:], bnar[:, 2 * c:2 * c + 1], AF.Copy,
                                     scale=1.0 / NCORES)
                e2 = stFt.tile([128, 1], F32, tag="e2c")
                nc.scalar.activation(e2[:], bnar[:, 2 * c + 1:2 * c + 2],
                                     AF.Copy, scale=1.0 / NCORES)
                vg = stFt.tile([128, 1], F32, tag="vg")
                nc.vector.tensor_mul(vg[:], mg[:], mg[:])
                nc.vector.tensor_sub(vg[:], e2[:], vg[:])
                sdc = stFt.tile([128, 1], F32, tag="sdc")
                nc.scalar.activation(sdc[:], vg[:], AF.Sqrt, bias=epsP[:])
                rs = stFt.tile([128, 1], F32, tag="rsc")
                nc.vector.reciprocal(rs[:], sdc[:])
                s1 = stFt.tile([128, 1], F32, tag="s1c")
                nc.vector.tensor_mul(s1[:], rs[:], bng_sb[:, c:c + 1])
                s2 = stFt.tile([128, 1], F32, tag="s2c")
                nc.vector.tensor_mul(s2[:], mg[:], s1[:])
                nc.vector.tensor_sub(s2[:], bnb_sb[:, c:c + 1], s2[:])
                sg2 = stFt.tile([128, HT], F32, tag="sg2")
                nc.scalar.activation(sg2[:], acc[:, c, :], AF.Sigmoid,
                                     scale=s1[:], bias=s2[:])
                yt = stFt.tile([128, HT], F32, tag="yt")
                nc.vector.tensor_scalar(yt[:], acc[:, c, :],
                                        s1[:], s2[:], ALU.mult, ALU.add)
                nc.vector.tensor_mul(ysl[:, c, :], yt[:], sg2[:])
            with tc.tile_pool(name="cvp2", bufs=2, space="PSUM") as cps2:
                for f in range(4):
                    psw = cps2.tile([128, HT], F32, tag="psw")
                    for c in range(4):
                        nc.tensor.matmul(psw[:], w2[:, c, ts(f, 128)],
                                         ysl[:, c, :],
                                         start=(c == 0), stop=(c == 3))
                    nc.vector.scalar_tensor_tensor(
                        out=h3_sb[:, f, :], in0=psw[:], scalar=bp2[:, f:f + 1],
                        in1=h2w[:, f, 16:16 + HT], op0=ALU.add, op1=ALU.add)
                    nc.vector.tensor_copy(h3_bf[:, f, :],
                                          h3_sb[:, f, :].bitcast(F32))

        # ================= Stage G: FFN2 =================
        with tc.tile_pool(name="stG", bufs=1) as stG, \
             tc.tile_pool(name="stGt", bufs=2) as stGt:
            rb4 = stG.tile([128, HT], F32)
            nb4 = stG.tile([128, HT], F32)
            emit_stats(h3_sb[:], 4, HT, [(0, HT)], rb4, nb4, 0, stGt,
                       _r(ones_c[:]))
            emit_ffn(h3_bf[:], h3_sb[:], rb4, nb4, h4_sb[:], None,
                     wg1_d, sg1_d, bg1_d, wg2_d, bg2_d, "f2")

        # ================= Stage H: LN5 + output =================
        with tc.tile_pool(name="stH", bufs=1) as stH, \
             tc.tile_pool(name="stHt", bufs=2) as stHt:
            g5_sb = stH.tile([128, 4], F32)
            nc.sync.dma_start(out=g5_sb[:], in_=g5_d[:])
            b5_sb = stH.tile([128, 4], F32)
            nc.sync.dma_start(out=b5_sb[:], in_=b5_d[:])
            rb5 = stH.tile([128, HT], F32)
            nb5 = stH.tile([128, HT], F32)
            emit_stats(h4_sb[:], 4, HT, [(0, HT)], rb5, nb5, 0, stHt,
                       _r(ones_c[:]))
            for c in range(4):
                u = stHt.tile([128, HT], F32, tag="h_u")
                nc.vector.tensor_mul(u[:], h4_sb[:, c, :], rb5[:])
                w_ = stHt.tile([128, HT], F32, tag="h_w")
                nc.vector.tensor_add(w_[:], u[:], nb5[:])
                xn5 = stHt.tile([128, HT], F32, tag="h_o")
                nc.vector.tensor_scalar(xn5[:], w_[:],
                                        g5_sb[:, c:c + 1], b5_sb[:, c:c + 1],
                                        ALU.mult, ALU.add)
                nc.sync.dma_start(out=out_d[c], in_=xn5[:])
    return nc


_CACHE = {}


def build_nc():
    if "nc" not in _CACHE:
        nc = bacc.Bacc("TRN2", target_bir_lowering=False, debug=False,
                       num_devices=NCORES)
        _emit(nc)
        nc.compile()
        _CACHE["nc"] = nc
    return _CACHE["nc"]


def _chunk_cf(a2d):
    """[Dany, W] -> [Dany//128, 128, W] chunk-major channels-first."""
    d, w = a2d.shape
    return np.ascontiguousarray(a2d.reshape(d // 128, 128, w), dtype=np.float32)


def to_bf16(a):
    import ml_dtypes
    return np.asarray(a, dtype=np.float32).astype(ml_dtypes.bfloat16)


def round_bf16(a):
    import ml_dtypes
    return np.asarray(a, dtype=np.float32).astype(
        ml_dtypes.bfloat16).astype(np.float32)


def _pcol(vec):
    """[Dout] per-channel vector -> [128, Dout//128] (partition, chunk)."""
    n = vec.shape[0]
    return np.ascontiguousarray(vec.reshape(n // 128, 128).T, dtype=np.float32)


def make_in_maps(inputs):
    inputs = {k: np.asarray(v, dtype=np.float32) for k, v in inputs.items()}
    x = inputs["x"]; pos_emb = inputs["pos_emb"]
    ln1_g, ln1_b = inputs["ln1_g"], inputs["ln1_b"]
    ln2_g, ln2_b = inputs["ln2_g"], inputs["ln2_b"]
    ln3_g, ln3_b = inputs["ln3_g"], inputs["ln3_b"]
    ln4_g, ln4_b = inputs["ln4_g"], inputs["ln4_b"]
    ln5_g, ln5_b = inputs["ln5_g"], inputs["ln5_b"]

    # FFN1/FFN2: W' = diag(g) W (bf16), b' = b + ln_b @ W, S = colsum(W')
    w1f = round_bf16(ln1_g[:, None] * inputs["ff1_w1"])
    b1f = inputs["ff1_b1"] + ln1_b @ inputs["ff1_w1"]
    s1f = w1f.sum(axis=0)
    w2f = round_bf16(0.5 * inputs["ff1_w2"]); b2f = 0.5 * inputs["ff1_b2"]
    wg1f = round_bf16(ln4_g[:, None] * inputs["ff2_w1"])
    bg1f = inputs["ff2_b1"] + ln4_b @ inputs["ff2_w1"]
    sg1f = wg1f.sum(axis=0)
    wg2f = round_bf16(0.5 * inputs["ff2_w2"]); bg2f = 0.5 * inputs["ff2_b2"]

    s = DK ** -0.5
    pos_u_f = inputs["pos_u"].reshape(D); pos_v_f = inputs["pos_v"].reshape(D)
    wqf = round_bf16(s * (ln2_g[:, None] * inputs["wq"]))
    bqf = s * (inputs["bq"] + ln2_b @ inputs["wq"] + pos_u_f)
    sqf = wqf.sum(axis=0)
    dqvf = s * (pos_v_f - pos_u_f)
    wkf = round_bf16(ln2_g[:, None] * inputs["wk"])
    bkf = inputs["bk"] + ln2_b @ inputs["wk"]
    skf = wkf.sum(axis=0)
    wvf = round_bf16(ln2_g[:, None] * inputs["wv"])
    bvf = inputs["bv"] + ln2_b @ inputs["wv"]
    svf = wvf.sum(axis=0)
    posT = np.zeros((D, 2048), dtype=np.float32)
    posT[:, :PB] = pos_emb[0].T

    pw1f = round_bf16((inputs["pw1_w"] * ln3_g[None, :]).T)    # [512, 1024]
    bpw1f = inputs["pw1_b"] + inputs["pw1_w"] @ ln3_b          # [1024]
    spwf = pw1f.sum(axis=0)
    dwwf = inputs["dw_w"][:, 0, :]                             # [512, 31]
    pw2f = round_bf16(inputs["pw2_w"].T)                       # [512, 512]

    base = {
        "wf1": to_bf16(_chunk_cf(w1f)), "sf1": _pcol(s1f), "bf1": _pcol(b1f),
        "wf2": to_bf16(_chunk_cf(w2f)), "bf2": _pcol(b2f),
        "wg1": to_bf16(_chunk_cf(wg1f)), "sg1": _pcol(sg1f),
        "bg1": _pcol(bg1f),
        "wg2": to_bf16(_chunk_cf(wg2f)), "bg2": _pcol(bg2f),
        "posT": to_bf16(_chunk_cf(posT)),
        "pw1": to_bf16(_chunk_cf(pw1f)), "spw": _pcol(spwf),
        "bpw1": _pcol(bpw1f),
        "dwv": np.ascontiguousarray(
            dwwf.reshape(4, 128, KCONV).transpose(1, 0, 2),
            dtype=np.float32),
        "bng": _pcol(inputs["bn_g"]), "bnb": _pcol(inputs["bn_b"]),
        "pw2": to_bf16(_chunk_cf(pw2f)), "bpw2": _pcol(inputs["pw2_b"]),
        "bo": _pcol(inputs["bo"]),
        "g5": _pcol(ln5_g), "b5": _pcol(ln5_b),
        "onc": np.ones((128, 1), dtype=np.float32),
        "onc_bf": to_bf16(np.ones((128, 1))),
        "idn": np.eye(128, dtype=np.float32),
    }

    in_maps = []
    for c in range(NCORES):
        b, scr = c // 2, c % 2
        cols = slice(256 * scr, 256 * scr + 256)
        m = dict(base)
        xb = x[b, scr * HT:(scr + 1) * HT, :].T               # [512, 512]
        m["x_loc"] = _chunk_cf(xb)
        m["x_bf"] = to_bf16(_chunk_cf(xb))
        m["wq"] = to_bf16(_chunk_cf(wqf[:, cols]))
        m["sq"] = _pcol(sqf[cols]); m["bq"] = _pcol(bqf[cols])
        m["dqv"] = _pcol(bqf[cols] + dqvf[cols])   # bqv = bq + dqv
        m["wk"] = to_bf16(_chunk_cf(wkf[:, cols]))
        m["sk"] = _pcol(skf[cols]); m["bk"] = _pcol(bkf[cols])
        m["wv"] = to_bf16(_chunk_cf(wvf[:, cols]))
        m["svrow"] = np.ascontiguousarray(svf[cols].reshape(1, 256),
                                          dtype=np.float32)
        m["bvrow"] = np.ascontiguousarray(bvf[cols].reshape(1, 256),
                                          dtype=np.float32)
        m["wp"] = to_bf16(_chunk_cf(inputs["wp"][:, cols]))
        wo_rows = inputs["wo"][cols, :]                       # [256, 512]
        m["wo"] = to_bf16(np.ascontiguousarray(wo_rows.reshape(4, 64, D)))
        cmask = np.ones((1, WIN), dtype=np.float32)
        if scr == 0:
            cmask[0, :16] = 0.0
        else:
            cmask[0, WIN - 16:] = 0.0
        m["cmask"] = cmask
        in_maps.append(m)
    return in_maps


def assemble_out(results):
    out = np.empty((B, T, D), dtype=np.float32)
    for c in range(NCORES):
        b, scr = c // 2, c % 2
        ol = np.asarray(results[c]["out_loc"])                # [4, 128, 512]
        out[b, scr * HT:(scr + 1) * HT, :] = ol.reshape(D, HT).T
    return out


def kernel(**inputs):
    in_maps = make_in_maps(inputs)
    nc = build_nc()
    res = run_bass_kernel_spmd(nc, in_maps, list(range(NCORES)))
    return assemble_out(res.results)


# revision 47
# speedup vs baseline: 1.0844x; 1.0078x over previous
"""Conformer layer on 8 Trainium2 NeuronCores (v2).

Sharding: core c handles batch b=c//2. Within a batch pair:
 - token-parallel (halves of T=1024) for FFN1/conv/FFN2/LN stages,
 - head-parallel (4 heads each) for attention.

v2 changes vs baseline:
 - LayerNorm fold: matmuls consume raw activations; per-token scale/shift is
   applied as a post-matmul fixup z = psy*rec + colsum(W')*(-m*rec) + b', so
   the PE never waits for LN statistics (keeps the HAM clock warm).
 - bf16 operands for attention (q/k/v/p/probs/o_h) with moving-dim-1024
   matmuls, bf16 FFN/pw/wo weights (same PE rate, half the DMA).
 - AllGather carries bf16 h and overlaps with p-projection + local-half QKV;
   a warmup collective at kernel start absorbs the first-cc latency.
 - Attention ReduceScatter in bf16.
 - Depthwise-conv diagonal matrices built on-chip (saves an 8MB DMA).
 - Plain loads on HWDGE (nc.sync), freeing GpSimd for casts/broadcasts.
"""

import numpy as np

import concourse.bass as bass
import concourse.mybir as mybir
import concourse.tile as tile
from concourse import bacc
from concourse.bass import ds, ts
from concourse.bass_utils import run_bass_kernel_spmd
from contextlib import ExitStack

F32 = mybir.dt.float32
F32R = mybir.dt.float32r
BF16 = mybir.dt.bfloat16
AF = mybir.ActivationFunctionType
ALU = mybir.AluOpType

D, DFF, H, DK, KCONV = 512, 2048, 8, 64, 31
B, T = 4, 1024
EPS = 1e-5
HT = 512            # tokens per core
WIN = 544           # conv window: 16 + 512 + 16
PB = 2047
BAND = 1152         # bd band width per q-chunk
BST = 1160          # bd dram row stride (elements)
NCORES = 8

PAIRS = [[0, 1], [2, 3], [4, 5], [6, 7]]
ALLG = [[0, 1, 2, 3, 4, 5, 6, 7]]


def _r(ap):
    return ap.bitcast(F32R)


def _emit(nc):
    def inp(name, shape, dt=F32):
        return nc.dram_tensor(name, list(shape), dt, kind="ExternalInput")

    x_d = inp("x_loc", (4, 128, HT), F32R)
    xbf_d = inp("x_bf", (4, 128, HT), BF16)
    wf1_d = inp("wf1", (4, 128, DFF), BF16)
    sf1_d = inp("sf1", (128, 16)); bf1_d = inp("bf1", (128, 16))
    wf2_d = inp("wf2", (16, 128, D), BF16); bf2_d = inp("bf2", (128, 4))
    wq_d = inp("wq", (4, 128, 256), BF16)
    sq_d = inp("sq", (128, 2)); bq_d = inp("bq", (128, 2))
    dqv_d = inp("dqv", (128, 2))
    wk_d = inp("wk", (4, 128, 256), BF16)
    sk_d = inp("sk", (128, 2)); bk_d = inp("bk", (128, 2))
    wv_d = inp("wv", (4, 128, 256), BF16)
    sv_d = inp("svrow", (1, 256)); bv_d = inp("bvrow", (1, 256))
    wp_d = inp("wp", (4, 128, 256), BF16)
    wo_d = inp("wo", (4, 64, D), BF16); bo_d = inp("bo", (128, 4))
    posT_d = inp("posT", (4, 128, 2048), BF16)
    pw1_d = inp("pw1", (4, 128, 1024), BF16)
    spw_d = inp("spw", (128, 8)); bpw1_d = inp("bpw1", (128, 8))
    dwv_d = inp("dwv", (128, 4, KCONV))
    bng_d = inp("bng", (128, 4)); bnb_d = inp("bnb", (128, 4))
    pw2_d = inp("pw2", (4, 128, D), BF16); bpw2_d = inp("bpw2", (128, 4))
    cmask_d = inp("cmask", (1, WIN))
    wg1_d = inp("wg1", (4, 128, DFF), BF16)
    sg1_d = inp("sg1", (128, 16)); bg1_d = inp("bg1", (128, 16))
    wg2_d = inp("wg2", (16, 128, D), BF16); bg2_d = inp("bg2", (128, 4))
    g5_d = inp("g5", (128, 4)); b5_d = inp("b5", (128, 4))
    onc_d = inp("onc", (128, 1), F32R)
    oncb_d = inp("onc_bf", (128, 1), BF16)
    idn_d = inp("idn", (128, 128))

    out_d = nc.dram_tensor("out_loc", [4, 128, HT], F32, kind="ExternalOutput")

    cc_w_in = nc.dram_tensor("cc_w_in", [1, 64], F32)
    cc_w_out = nc.dram_tensor("cc_w_out", [2, 64], F32)
    cc_h_in = nc.dram_tensor("cc_h_in", [4, 128, HT], F32)
    cc_h_out = nc.dram_tensor("cc_h_out", [2, 4, 128, HT], F32)
    cc_a_in = nc.dram_tensor("cc_a_in", [2, 4, 128, WIN], F32)
    cc_a_out = nc.dram_tensor("cc_a_out", [4, 128, WIN], F32)
    cc_bn_in = nc.dram_tensor("cc_bn_in", [128, 8], F32)
    cc_bn_out = nc.dram_tensor("cc_bn_out", [128, 8], F32)
    bd_d = [nc.dram_tensor(f"bd_{i}", [128 * BST], BF16) for i in range(32)]

    uid = [0]

    with tile.TileContext(nc) as tc, ExitStack() as ctx:
        const = ctx.enter_context(tc.tile_pool(name="const", bufs=1))
        ones_c = const.tile([128, 1], F32R)
        nc.sync.dma_start(out=ones_c[:], in_=onc_d[:])
        ones_cb = const.tile([128, 1], BF16)
        nc.sync.dma_start(out=ones_cb[:], in_=oncb_d[:])
        eps1 = const.tile([1, 1], F32); nc.vector.memset(eps1[:], EPS)
        epsP = const.tile([128, 1], F32); nc.vector.memset(epsP[:], EPS)
        identb = const.tile([128, 128], BF16)
        nc.gpsimd.dma_start(out=identb[:], in_=idn_d[:])

        # CC-stream warmup: tiny 8-way AllReduce absorbs the bootstrap
        # barrier + cross-core startup skew while FFN1 runs.
        warm = const.tile([1, 64], F32)
        nc.vector.memset(warm[:], 0.0)
        nc.gpsimd.dma_start(out=cc_w_in[:], in_=warm[:])
        nc.gpsimd.collective_compute(
            "AllReduce", ALU.add, ins=[cc_w_in[:]], outs=[cc_w_out[0:1]],
            replica_groups=ALLG)

        # per-engine copies of the core's token-half index (register values
        # are engine-local)
        pidv = nc.vector.partition_id()
        scv, scv2 = pidv % 2, (pidv + 1) % 2
        pida = nc.scalar.partition_id()
        sca, sca2 = pida % 2, (pida + 1) % 2
        pids = nc.sync.partition_id()
        scs, scs2 = pids % 2, (pids + 1) % 2

        act = ctx.enter_context(tc.tile_pool(name="act", bufs=1))
        hfe_sb = act.tile([128, 4, WIN], F32R)   # conv residual window
        h_sb = act.tile([128, 4, HT], F32R)      # post-FFN1 hidden (local)
        h_bf = act.tile([128, 4, HT], BF16)
        h3_sb = act.tile([128, 4, HT], F32R)     # post-conv hidden
        h3_bf = act.tile([128, 4, HT], BF16)
        h4_sb = act.tile([128, 4, HT], F32R)     # post-FFN2 hidden

        # ---------- LN statistics (fold form) ----------
        # Produces broadcast tiles RB = 1/std and NM = -mean/std per token.
        def emit_stats(x4, nchunk, W, blocks, rb_t, nb_t, col0, sbp, ones):
            uid[0] += 1
            with tc.tile_pool(name=f"lnps{uid[0]}", bufs=1,
                              space="PSUM") as lnps:
                x2 = sbp.tile([128, nchunk, W], F32R, tag="ln_sq")
                nc.scalar.square(x2[:], x4)
                for b0, bw in blocks:
                    pss = lnps.tile([1, bw], F32, tag="lns")
                    psq = lnps.tile([1, bw], F32, tag="lnq")
                    for c in range(nchunk):
                        nc.tensor.matmul(pss[:], ones,
                                         x4[:, c, b0:b0 + bw],
                                         start=(c == 0), stop=(c == nchunk - 1))
                    for c in range(nchunk):
                        nc.tensor.matmul(psq[:], _r(ones_c[:]),
                                         _r(x2[:, c, b0:b0 + bw]),
                                         start=(c == 0), stop=(c == nchunk - 1))
                    mng = sbp.tile([1, bw], F32, tag="ln_m")
                    nc.scalar.activation(mng[:], pss[:], AF.Copy,
                                         scale=-1.0 / D)
                    e2 = sbp.tile([1, bw], F32, tag="ln_e2")
                    nc.scalar.activation(e2[:], psq[:], AF.Copy, scale=1.0 / D)
                    var = sbp.tile([1, bw], F32, tag="ln_var")
                    nc.vector.tensor_mul(var[:], mng[:], mng[:])
                    nc.vector.tensor_sub(var[:], e2[:], var[:])
                    sd = sbp.tile([1, bw], F32, tag="ln_sd")
                    nc.scalar.activation(sd[:], var[:], AF.Sqrt, bias=eps1[:])
                    rec = sbp.tile([1, bw], F32, tag="ln_rs")
                    scr = sbp.tile([1, bw], F32, tag="ln_scr")
                    nc.vector.reciprocal_approx_accurate(rec[:], sd[:], scr[:])
                    nmr = sbp.tile([1, bw], F32, tag="ln_nm")
                    nc.vector.tensor_mul(nmr[:], mng[:], rec[:])
                    nc.gpsimd.partition_broadcast(
                        rb_t[:, col0 + b0:col0 + b0 + bw], rec[:])
                    nc.gpsimd.partition_broadcast(
                        nb_t[:, col0 + b0:col0 + b0 + bw], nmr[:])

        # ---------- FFN with LN fold ----------
        def emit_ffn(xbf, xres, rb_t, nb_t, out, outbf, w1dram, s1dram,
                     b1dram, w2dram, b2dram, pref):
            with tc.tile_pool(name=pref + "w", bufs=1) as wp_, \
                 tc.tile_pool(name=pref + "t", bufs=3) as tp, \
                 tc.tile_pool(name=pref + "ps", bufs=2, space="PSUM") as psp, \
                 tc.tile_pool(name=pref + "ph", bufs=1, space="PSUM") as php:
                w1 = wp_.tile([128, 4, DFF], BF16)
                for c in range(4):
                    nc.sync.dma_start(out=w1[:, c, :], in_=w1dram[c])
                w2 = wp_.tile([128, 16, D], BF16)
                for j in range(16):
                    nc.sync.dma_start(out=w2[:, j, :], in_=w2dram[j])
                s1 = wp_.tile([128, 16], F32)
                nc.sync.dma_start(out=s1[:], in_=s1dram[:])
                b1 = wp_.tile([128, 16], F32)
                nc.sync.dma_start(out=b1[:], in_=b1dram[:])
                b2 = wp_.tile([128, 4], F32)
                nc.sync.dma_start(out=b2[:], in_=b2dram[:])
                psh = php.tile([128, 4, HT], F32)
                for j in range(16):
                    psy = psp.tile([128, HT], F32, tag="psy")
                    for c in range(4):
                        nc.tensor.matmul(psy[:], w1[:, c, ts(j, 128)],
                                         xbf[:, c, :],
                                         start=(c == 0), stop=(c == 3))
                    u = tp.tile([128, HT], F32, tag="u")
                    nc.vector.tensor_mul(u[:], psy[:], rb_t[:])
                    z = tp.tile([128, HT], F32, tag="z")
                    nc.vector.scalar_tensor_tensor(
                        out=z[:], in0=nb_t[:], scalar=s1[:, j:j + 1],
                        in1=u[:], op0=ALU.mult, op1=ALU.add)
                    sig = tp.tile([128, HT], F32, tag="sig")
                    nc.scalar.activation(sig[:], z[:], AF.Sigmoid,
                                         bias=b1[:, j:j + 1])
                    y1 = tp.tile([128, HT], BF16, tag="y1")
                    nc.vector.scalar_tensor_tensor(
                        out=y1[:], in0=z[:], scalar=b1[:, j:j + 1],
                        in1=sig[:], op0=ALU.add, op1=ALU.mult)
                    for f in range(4):
                        nc.tensor.matmul(psh[:, f, :],
                                         w2[:, j, ts(f, 128)], y1[:],
                                         start=(j == 0), stop=(j == 15))
                for c in range(4):
                    nc.vector.scalar_tensor_tensor(
                        out=out[:, c, :], in0=psh[:, c, :],
                        scalar=b2[:, c:c + 1], in1=xres[:, c, :],
                        op0=ALU.add, op1=ALU.add)
                    if outbf is not None:
                        nc.vector.tensor_copy(outbf[:, c, :],
                                              out[:, c, :].bitcast(F32))

        # ================= Stage A: FFN1 on local tokens =================
        with tc.tile_pool(name="stA", bufs=1) as stA, \
             tc.tile_pool(name="stAt", bufs=2) as stAt:
            x_sb = stA.tile([128, 4, HT], F32R)
            for c in range(4):
                nc.gpsimd.dma_start(out=x_sb[:, c, :], in_=x_d[c])
            x_bf = stA.tile([128, 4, HT], BF16)
            for c in range(4):
                nc.sync.dma_start(out=x_bf[:, c, :], in_=xbf_d[c])
            rb1 = stA.tile([128, HT], F32)
            nb1 = stA.tile([128, HT], F32)
            emit_stats(x_sb[:], 4, HT, [(0, HT)], rb1, nb1, 0, stAt,
                       _r(ones_c[:]))
            emit_ffn(x_bf[:], x_sb[:], rb1, nb1, h_sb[:], h_bf[:],
                     wf1_d, sf1_d, bf1_d, wf2_d, bf2_d, "f1")
            # conv residual window: local half occupies [16, 528)
            for c in range(4):
                nc.vector.tensor_copy(hfe_sb[:, c, 16:16 + HT].bitcast(F32),
                                      h_sb[:, c, :].bitcast(F32))
            for c in range(4):
                nc.sync.dma_start(out=cc_h_in[c],
                                  in_=h_sb[:, c, :].bitcast(F32))

        nc.gpsimd.collective_compute(
            "AllGather", ALU.bypass, ins=[cc_h_in[:]], outs=[cc_h_out[:]],
            replica_groups=PAIRS)

        # ============ attention scope ============
        with tc.tile_pool(name="attp", bufs=1) as attp:
            woff = sca * HT                # local half start (ACT offsets)
            roff = sca2 * HT               # remote half start

            q_sb = attp.tile([128, 2, T], BF16)
            qv_sb = attp.tile([128, 2, T], BF16)
            k_sb = attp.tile([128, 2, T], BF16)
            v65 = attp.tile([128, 8, 4, 65], BF16)
            nc.vector.memset(v65[:, :, :, 64:65], 1.0)
            p_sb = attp.tile([128, 2, 2048], BF16)
            # rb2/nb2 and recc/nmc are stored local-half-first (static
            # writes); the eviction target offsets stay symbolic.
            rb2 = attp.tile([128, T], F32)
            nb2 = attp.tile([128, T], F32)
            recc = attp.tile([128, 8], F32)
            nmc = attp.tile([128, 8], F32)

            # ---- p-projection (overlaps AllGather) ----
            with tc.tile_pool(name="ppp", bufs=2, space="PSUM") as ppp, \
                 tc.tile_pool(name="atw", bufs=1) as atw:
                wp_sb = atw.tile([128, 4, 256], BF16, tag="wp")
                for c in range(4):
                    nc.sync.dma_start(out=wp_sb[:, c, :], in_=wp_d[c])
                pos_sb = atw.tile([128, 4, 2048], BF16, tag="pos")
                for c in range(4):
                    nc.sync.dma_start(out=pos_sb[:, c, :], in_=posT_d[c])
                for m in range(2):
                    for pc in range(4):
                        psp_t = ppp.tile([128, 512], F32, tag="psp")
                        for c in range(4):
                            nc.tensor.matmul(
                                psp_t[:], wp_sb[:, c, ts(m, 128)],
                                pos_sb[:, c, ts(pc, 512)],
                                start=(c == 0), stop=(c == 3))
                        nc.scalar.activation(p_sb[:, m, ts(pc, 512)],
                                             psp_t[:], AF.Copy)

            # ---- local-half LN2 stats + qkv (overlap AllGather) ----
            with tc.tile_pool(name="stC", bufs=1) as stC, \
                 tc.tile_pool(name="stCt", bufs=2) as stCt, \
                 tc.tile_pool(name="qkp", bufs=2, space="PSUM") as qkps:

                def emit_stats2(x4, ones, rcol):
                    # LN2 stats for one half; writes rb2/nb2[:, rcol:rcol+HT]
                    # and recc/nmc[:, rcol//128 : +4] (static columns).
                    uid[0] += 1
                    cc = rcol // 128
                    with tc.tile_pool(name=f"lnps{uid[0]}", bufs=1,
                                      space="PSUM") as lnps:
                        x2 = stCt.tile([128, 4, HT], F32R, tag="ln_sq")
                        nc.scalar.square(x2[:], x4)
                        pss = lnps.tile([1, HT], F32, tag="lns")
                        psq = lnps.tile([1, HT], F32, tag="lnq")
                        for c in range(4):
                            nc.tensor.matmul(pss[:], ones, x4[:, c, :],
                                             start=(c == 0), stop=(c == 3))
                        for c in range(4):
                            nc.tensor.matmul(psq[:], _r(ones_c[:]),
                                             _r(x2[:, c, :]),
                                             start=(c == 0), stop=(c == 3))
                        mng = stCt.tile([1, HT], F32, tag="ln_m")
                        nc.scalar.activation(mng[:], pss[:], AF.Copy,
                                             scale=-1.0 / D)
                        e2 = stCt.tile([1, HT], F32, tag="ln_e2")
                        nc.scalar.activation(e2[:], psq[:], AF.Copy,
                                             scale=1.0 / D)
                        var = stCt.tile([1, HT], F32, tag="ln_var")
                        nc.vector.tensor_mul(var[:], mng[:], mng[:])
                        nc.vector.tensor_sub(var[:], e2[:], var[:])
                        sd = stCt.tile([1, HT], F32, tag="ln_sd")
                        nc.scalar.activation(sd[:], var[:], AF.Sqrt,
                                             bias=eps1[:])
                        rec2 = stCt.tile([1, HT], F32, tag="ln_rec")
                        scr = stCt.tile([1, HT], F32, tag="ln_scr")
                        nc.vector.reciprocal_approx_accurate(rec2[:], sd[:],
                                                             scr[:])
                        nm2 = stCt.tile([1, HT], F32, tag="ln_nm")
                        nc.vector.tensor_mul(nm2[:], mng[:], rec2[:])
                        nc.gpsimd.partition_broadcast(
                            rb2[:, rcol:rcol + HT], rec2[:])
                        nc.gpsimd.partition_broadcast(
                            nb2[:, rcol:rcol + HT], nm2[:])
                        for b in range(4):
                            nc.gpsimd.dma_start(
                                out=recc[:, cc + b:cc + b + 1],
                                in_=rec2[:, ts(b, 128)])
                            nc.gpsimd.dma_start(
                                out=nmc[:, cc + b:cc + b + 1],
                                in_=nm2[:, ts(b, 128)])

                wq_sb = stC.tile([128, 4, 256], BF16)
                wk_sb = stC.tile([128, 4, 256], BF16)
                wv_sb = stC.tile([128, 4, 256], BF16)
                for c in range(4):
                    nc.sync.dma_start(out=wq_sb[:, c, :], in_=wq_d[c])
                    nc.sync.dma_start(out=wk_sb[:, c, :], in_=wk_d[c])
                    nc.sync.dma_start(out=wv_sb[:, c, :], in_=wv_d[c])
                sq_sb = stC.tile([128, 2], F32)
                nc.sync.dma_start(out=sq_sb[:], in_=sq_d[:])
                bq_sb = stC.tile([128, 2], F32)
                nc.sync.dma_start(out=bq_sb[:], in_=bq_d[:])
                sk_sb = stC.tile([128, 2], F32)
                nc.sync.dma_start(out=sk_sb[:], in_=sk_d[:])
                bk_sb = stC.tile([128, 2], F32)
                nc.sync.dma_start(out=bk_sb[:], in_=bk_d[:])
                bqv_sb = stC.tile([128, 2], F32)
                nc.sync.dma_start(out=bqv_sb[:], in_=dqv_d[:])
                sv_row = stC.tile([1, 256], F32)
                nc.sync.dma_start(out=sv_row[:], in_=sv_d[:])
                svb = stC.tile([128, 256], F32)
                nc.gpsimd.partition_broadcast(svb[:], sv_row[:])
                bv_row = stC.tile([1, 256], F32)
                nc.sync.dma_start(out=bv_row[:], in_=bv_d[:])
                bvb = stC.tile([128, 256], F32)
                nc.gpsimd.partition_broadcast(bvb[:], bv_row[:])

                def emit_qk(xin, rcol, off):
                    # rcol: static column base in rb2/nb2 (local-first);
                    # off: symbolic global token offset for the outputs.
                    for m in range(2):
                        for w_sb, s_w, b_w, dst in (
                                (wq_sb, sq_sb, bq_sb, q_sb),
                                (wk_sb, sk_sb, bk_sb, k_sb)):
                            ps = qkps.tile([128, HT], F32, tag="psqk")
                            for c in range(4):
                                nc.tensor.matmul(
                                    ps[:], w_sb[:, c, ts(m, 128)],
                                    xin[:, c, :],
                                    start=(c == 0), stop=(c == 3))
                            u = stCt.tile([128, HT], F32, tag="qk_u")
                            nc.vector.tensor_mul(u[:], ps[:],
                                                 rb2[:, rcol:rcol + HT])
                            w_ = stCt.tile([128, HT], F32, tag="qk_w")
                            nc.vector.scalar_tensor_tensor(
                                out=w_[:], in0=nb2[:, rcol:rcol + HT],
                                scalar=s_w[:, m:m + 1], in1=u[:],
                                op0=ALU.mult, op1=ALU.add)
                            nc.scalar.activation(
                                dst[:, m, ds(off, HT)], w_[:], AF.Identity,
                                bias=b_w[:, m:m + 1])
                            if dst is q_sb:
                                nc.scalar.activation(
                                    qv_sb[:, m, ds(off, HT)], w_[:],
                                    AF.Identity, bias=bqv_sb[:, m:m + 1])

                def emit_v(xin, vbase, rbase):
                    # vbase: symbolic global chunk base; rbase: static
                    # column base into recc/nmc.
                    for tq in range(4):
                        psv = qkps.tile([128, 256], F32, tag="psv")
                        for c in range(4):
                            nc.tensor.matmul(
                                psv[:], xin[:, c, ts(tq, 128)],
                                wv_sb[:, c, :],
                                start=(c == 0), stop=(c == 3))
                        u = stCt.tile([128, 256], F32, tag="v_u")
                        nc.vector.tensor_scalar_mul(
                            u[:], psv[:], recc[:, rbase + tq:rbase + tq + 1])
                        w_ = stCt.tile([128, 256], F32, tag="v_w")
                        nc.vector.scalar_tensor_tensor(
                            out=w_[:], in0=svb[:],
                            scalar=nmc[:, rbase + tq:rbase + tq + 1],
                            in1=u[:], op0=ALU.mult, op1=ALU.add)
                        vt = stCt.tile([128, 256], BF16, tag="v_t")
                        nc.vector.tensor_add(vt[:], w_[:], bvb[:])
                        nc.vector.tensor_copy(
                            v65[:, ds(vbase + tq, 1), :, 0:64]
                            .rearrange("p o h d -> p (o h) d"),
                            vt[:].rearrange("p (h d) -> p h d", h=4))

                emit_stats2(h_sb[:], _r(ones_c[:]), 0)
                emit_qk(h_bf[:], 0, woff)
                emit_v(h_bf[:], scv * 4, 0)

                # ---- gather lands: remote half ----
                h_rem = stC.tile([128, 4, HT], F32R)
                for c in range(4):
                    nc.sync.dma_start(
                        out=h_rem[:, c, :].bitcast(F32),
                        in_=cc_h_out[:].rearrange("s c p t -> (s c) p t")
                        [ds(scs2 * 4 + c, 1)]
                        .rearrange("o p t -> (o p) t"))
                h_rem_bf = stC.tile([128, 4, HT], BF16)
                for c in range(4):
                    nc.vector.tensor_copy(h_rem_bf[:, c, :],
                                          h_rem[:, c, :].bitcast(F32))
                # conv halo: zero edges then copy 16 remote tokens
                nc.vector.memset(hfe_sb[:, :, 0:16].bitcast(F32), 0.0)
                nc.vector.memset(hfe_sb[:, :, 16 + HT:].bitcast(F32), 0.0)
                for c in range(4):
                    nc.vector.tensor_copy(
                        hfe_sb[:, c, ds(scv2 * (16 + HT), 16)].bitcast(F32),
                        h_rem[:, c, ds(scv * (HT - 16), 16)].bitcast(F32))

                emit_stats2(h_rem[:], _r(ones_c[:]), HT)
                emit_qk(h_rem_bf[:], HT, roff)
                emit_v(h_rem_bf[:], scv2 * 4, 4)

            # ---- bd banded matmuls for all heads ----
            with tc.tile_pool(name="bds", bufs=2) as bdp, \
                 tc.tile_pool(name="psb", bufs=2, space="PSUM") as psb:
                for h in range(4):
                    hc, hr = h // 2, 64 * (h % 2)
                    for qc in range(8):
                        base = max(0, 895 - 128 * qc)
                        bdw = bdp.tile([128, BAND], BF16, tag="bdw")
                        for pi in range(3):
                            psB = psb.tile([128, 384], F32, tag="psB")
                            nc.tensor.matmul(
                                psB[:],
                                qv_sb[hr:hr + 64, hc, ts(qc, 128)],
                                p_sb[hr:hr + 64, hc,
                                     base + pi * 384: base + (pi + 1) * 384],
                                start=True, stop=True)
                            nc.scalar.activation(
                                bdw[:, pi * 384:(pi + 1) * 384], psB[:],
                                AF.Copy)
                        nc.sync.dma_start(
                            out=bd_d[h * 8 + qc][:].rearrange(
                                "(p w) -> p w", p=128)[:, 0:BAND],
                            in_=bdw[:])

            # ---- attention heads: scores + softmax + AV ----
            o_h = [attp.tile([64, T], BF16, name=f"o_h{_h}", tag=f"o_h{_h}")
                   for _h in range(4)]
            with tc.tile_pool(name="bdsh", bufs=1) as shp, \
                 tc.tile_pool(name="atp", bufs=3) as atp, \
                 tc.tile_pool(name="atn", bufs=1) as atn, \
                 tc.tile_pool(name="pss", bufs=2, space="PSUM") as pss_p, \
                 tc.tile_pool(name="psav", bufs=2, space="PSUM") as psav:
                for h in range(4):
                    hc, hr = h // 2, 64 * (h % 2)
                    # XBAR transpose during the shifted read:
                    # bdshT[p, kc, q] = bd[q, kc*128+p]  (k-major layout)
                    bdshT = shp.tile([128, 8, T], BF16, tag=f"sh{h % 2}")
                    for qc in range(8):
                        c0 = 127 if qc == 7 else 128
                        src = bass.AP(tensor=bd_d[h * 8 + qc], offset=c0,
                                      ap=[[BST - 1, 128], [1, T]])
                        nc.sync.dma_start(
                            out=bdshT[:, :, ts(qc, 128)], in_=src,
                            transpose=True)
                    psA = psav.tile([65, T], F32, tag="psAV")
                    for kc in range(8):
                        psS = pss_p.tile([128, T], F32, tag="psS")
                        for th in range(2):
                            nc.tensor.matmul(
                                psS[:, th * 512:(th + 1) * 512],
                                k_sb[hr:hr + 64, hc, ts(kc, 128)],
                                q_sb[hr:hr + 64, hc,
                                     th * 512:(th + 1) * 512],
                                start=True, stop=False, skip_group_check=True)
                        for th in range(2):
                            nc.tensor.matmul(
                                psS[:, th * 512:(th + 1) * 512],
                                identb[:],
                                bdshT[:, kc, th * 512:(th + 1) * 512],
                                start=False, stop=True,
                                skip_group_check=True)
                        probs = atp.tile([128, T], BF16, tag="probs")
                        nc.scalar.activation(probs[:], psS[:], AF.Exp)
                        for th in range(2):
                            nc.tensor.matmul(
                                psA[:, th * 512:(th + 1) * 512],
                                v65[:, kc, h, :],
                                probs[:, th * 512:(th + 1) * 512],
                                start=(kc == 0), stop=(kc == 7),
                                skip_group_check=True)
                    s65 = atn.tile([65, T], F32, tag="s65")
                    nc.vector.tensor_copy(s65[64:65, :], psA[64:65, :])
                    row = atn.tile([1, T], F32, tag="row")
                    nc.gpsimd.dma_start(out=row[:], in_=s65[64:65, :])
                    rec = atn.tile([1, T], F32, tag="rec")
                    scr = atn.tile([1, T], F32, tag="scr")
                    nc.vector.reciprocal_approx_accurate(rec[:], row[:],
                                                         scr[:])
                    rb = atn.tile([64, T], F32, tag="rb")
                    nc.gpsimd.partition_broadcast(rb[:], rec[:])
                    nc.vector.tensor_mul(o_h[h][:], psA[0:64, :], rb[:])

            # ---- out-projection + pair ReduceScatter (bf16) ----
            with tc.tile_pool(name="pso", bufs=2, space="PSUM") as psop, \
                 tc.tile_pool(name="aot", bufs=2) as aot:
                wo_sb = aot.tile([64, 4, D], BF16, tag="wo", bufs=1)
                for hh in range(4):
                    nc.sync.dma_start(out=wo_sb[:, hh, :], in_=wo_d[hh])
                for f in range(4):
                    pso = psop.tile([128, T], F32, tag="pso")
                    for th in range(2):
                        for hh in range(4):
                            nc.tensor.matmul(
                                pso[:, th * 512:(th + 1) * 512],
                                wo_sb[:, hh, ts(f, 128)],
                                o_h[hh][:, th * 512:(th + 1) * 512],
                                start=(hh == 0), stop=(hh == 3),
                                skip_group_check=True)
                    ao = aot.tile([128, 2, WIN], F32, tag="ao")
                    nc.vector.memset(ao[:, 0, 0:16], 0.0)
                    nc.vector.memset(ao[:, 1, WIN - 16:WIN], 0.0)
                    nc.vector.tensor_copy(ao[:, 0, 16:WIN], pso[:, 0:528])
                    nc.vector.tensor_copy(ao[:, 1, 0:528], pso[:, 496:T])
                    nc.sync.dma_start(out=cc_a_in[0, f], in_=ao[:, 0, :])
                    nc.sync.dma_start(out=cc_a_in[1, f], in_=ao[:, 1, :])

        nc.gpsimd.collective_compute(
            "ReduceScatter", ALU.add, ins=[cc_a_in[:]], outs=[cc_a_out[:]],
            replica_groups=PAIRS)

        # ================= Stage F: conv module =================
        with tc.tile_pool(name="stF", bufs=1) as stF, \
             tc.tile_pool(name="stFt", bufs=2) as stFt:
            # on-chip depthwise diag build (overlaps ReduceScatter)
            dwv = stF.tile([128, 4, KCONV], F32)
            nc.sync.dma_start(out=dwv[:], in_=dwv_d[:])
            dg = stF.tile([128, 4, KCONV, 128], BF16)
            for c in range(4):
                for j in range(KCONV):
                    nc.vector.tensor_scalar(
                        dg[:, c, j, :], identb[:],
                        dwv[:, c, j:j + 1], 0.0, ALU.mult, ALU.add)
            w1 = stF.tile([128, 4, 1024], BF16)
            for c in range(4):
                nc.sync.dma_start(out=w1[:, c, :], in_=pw1_d[c])
            spw = stF.tile([128, 8], F32)
            nc.sync.dma_start(out=spw[:], in_=spw_d[:])
            bp1 = stF.tile([128, 8], F32)
            nc.sync.dma_start(out=bp1[:], in_=bpw1_d[:])
            w2 = stF.tile([128, 4, D], BF16)
            for c in range(4):
                nc.sync.dma_start(out=w2[:, c, :], in_=pw2_d[c])
            bp2 = stF.tile([128, 4], F32)
            nc.sync.dma_start(out=bp2[:], in_=bpw2_d[:])
            bo_sb = stF.tile([128, 4], F32)
            nc.sync.dma_start(out=bo_sb[:], in_=bo_d[:])
            cm = stF.tile([1, WIN], F32)
            nc.sync.dma_start(out=cm[:], in_=cmask_d[:])
            cmb = stF.tile([128, WIN], F32)
            nc.gpsimd.partition_broadcast(cmb[:], cm[:])

            h2w = stF.tile([128, 4, WIN], F32R)
            for c in range(4):
                at = stFt.tile([128, WIN], F32, tag="at")
                nc.sync.dma_start(out=at[:], in_=cc_a_out[c])
                nc.vector.scalar_tensor_tensor(
                    out=h2w[:, c, :], in0=at[:],
                    scalar=bo_sb[:, c:c + 1],
                    in1=hfe_sb[:, c, :].bitcast(F32),
                    op0=ALU.add, op1=ALU.add)
            h2w_bf = stF.tile([128, 4, WIN], BF16)
            for c in range(4):
                nc.vector.tensor_copy(h2w_bf[:, c, :],
                                      h2w[:, c, :].bitcast(F32))

            rb3 = stF.tile([128, WIN], F32)
            nb3 = stF.tile([128, WIN], F32)
            emit_stats(h2w[:], 4, WIN, [(0, 272), (272, 272)], rb3, nb3, 0,
                       stFt, _r(ones_c[:]))

            glu = stF.tile([128, 4, WIN], BF16)
            with tc.tile_pool(name="cvp1", bufs=1, space="PSUM") as cps:
                for m in range(4):
                    psa = cps.tile([128, 2, 512], F32, tag="psa")
                    psg = cps.tile([128, 2, 512], F32, tag="psg")
                    for half in range(2):
                        sl = slice(half * 272, (half + 1) * 272)
                        for c in range(4):
                            nc.tensor.matmul(psa[:, half, 0:272],
                                             w1[:, c, ts(m, 128)],
                                             h2w_bf[:, c, sl],
                                             start=(c == 0), stop=(c == 3),
                                             skip_group_check=True)
                        for c in range(4):
                            nc.tensor.matmul(psg[:, half, 0:272],
                                             w1[:, c, 512 + m * 128:
                                                 512 + (m + 1) * 128],
                                             h2w_bf[:, c, sl],
                                             start=(c == 0), stop=(c == 3),
                                             skip_group_check=True)
                    for half in range(2):
                        sl = slice(half * 272, (half + 1) * 272)
                        ua = stFt.tile([128, 272], F32, tag="cv_ua")
                        nc.vector.tensor_mul(ua[:], psa[:, half, 0:272],
                                             rb3[:, sl])
                        za = stFt.tile([128, 272], F32, tag="cv_za")
                        nc.vector.scalar_tensor_tensor(
                            out=za[:], in0=nb3[:, sl],
                            scalar=spw[:, m:m + 1], in1=ua[:],
                            op0=ALU.mult, op1=ALU.add)
                        ug = stFt.tile([128, 272], F32, tag="cv_ug")
                        nc.vector.tensor_mul(ug[:], psg[:, half, 0:272],
                                             rb3[:, sl])
                        zg = stFt.tile([128, 272], F32, tag="cv_zg")
                        nc.vector.scalar_tensor_tensor(
                            out=zg[:], in0=nb3[:, sl],
                            scalar=spw[:, 4 + m:5 + m], in1=ug[:],
                            op0=ALU.mult, op1=ALU.add)
                        sg = stFt.tile([128, 272], F32, tag="cv_sg")
                        nc.scalar.activation(sg[:], zg[:], AF.Sigmoid,
                                             bias=bp1[:, 4 + m:5 + m])
                        gl = stFt.tile([128, 272], F32, tag="cv_gl")
                        nc.vector.scalar_tensor_tensor(
                            out=gl[:], in0=za[:], scalar=bp1[:, m:m + 1],
                            in1=sg[:], op0=ALU.add, op1=ALU.mult)
                        nc.vector.tensor_mul(glu[:, m, sl], gl[:],
                                             cmb[:, sl])

            # depthwise conv: 31 accumulating diagonal matmuls per chunk
            acc = stF.tile([128, 4, HT], F32)
            with tc.tile_pool(name="dgp", bufs=2, space="PSUM") as dgp:
                for c in range(4):
                    psC = dgp.tile([128, HT], F32, tag="psC")
                    for j in range(KCONV):
                        nc.tensor.matmul(psC[:], dg[:, c, j, :],
                                         glu[:, c, 1 + j:1 + j + HT],
                                         start=(j == 0), stop=(j == KCONV - 1))
                    nc.vector.tensor_copy(acc[:, c, :], psC[:])
            # BN stats + 8-way AllReduce
            bnpack = stF.tile([128, 8], F32)
            for c in range(4):
                bst_t = stFt.tile([128, 6], F32, tag="bst")
                nc.vector.bn_stats(bst_t[:], acc[:, c, :])
                mv = stFt.tile([128, 2], F32, tag="mv")
                nc.vector.bn_aggr(mv[:], bst_t[:])
                nc.vector.tensor_copy(bnpack[:, 2 * c:2 * c + 1], mv[:, 0:1])
                nc.vector.scalar_tensor_tensor(
                    out=bnpack[:, 2 * c + 1:2 * c + 2], in0=mv[:, 0:1],
                    scalar=mv[:, 0:1], in1=mv[:, 1:2],
                    op0=ALU.mult, op1=ALU.add)
            nc.sync.dma_start(out=cc_bn_in[:], in_=bnpack[:])
            nc.gpsimd.collective_compute(
                "AllReduce", ALU.add, ins=[cc_bn_in[:]], outs=[cc_bn_out[:]],
                replica_groups=ALLG)
            bnar = stF.tile([128, 8], F32)
            nc.sync.dma_start(out=bnar[:], in_=cc_bn_out[:])
            bng_sb = stF.tile([128, 4], F32)
            nc.sync.dma_start(out=bng_sb[:], in_=bng_d[:])
            bnb_sb = stF.tile([128, 4], F32)
            nc.sync.dma_start(out=bnb_sb[:], in_=bnb_d[:])
            ysl = stF.tile([128, 4, HT], BF16)
            for c in range(4):
                mg = stFt.tile([128, 1], F32, tag="mg")
                nc.scalar.activation(mg[:], bnar[:, 2 * c:2 * c + 1], AF.Copy,
                                     scale=1.0 / NCORES)
                e2 = stFt.tile([128, 1], F32, tag="e2c")
                nc.scalar.activation(e2[:], bnar[:, 2 * c + 1:2 * c + 2],
                                     AF.Copy, scale=1.0 / NCORES)
                vg = stFt.tile([128, 1], F32, tag="vg")
                nc.vector.tensor_mul(vg[:], mg[:], mg[:])
                nc.vector.tensor_sub(vg[:], e2[:], vg[:])
                sdc = stFt.tile([128, 1], F32, tag="sdc")
                nc.scalar.activation(sdc[:], vg[:], AF.Sqrt, bias=epsP[:])
                rs = stFt.tile([128, 1], F32, tag="rsc")
                nc.vector.reciprocal(rs[:], sdc[:])
                s1 = stFt.tile([128, 1], F32, tag="s1c")
                nc.vector.tensor_mul(s1[:], rs[:], bng_sb[:, c:c + 1])
                s2 = stFt.tile([128, 1], F32, tag="s2c")
                nc.vector.tensor_mul(s2[:], mg[:], s1[:])
                nc.vector.tensor_sub(s2[:], bnb_sb[:, c:c + 1], s2[:])
                sg2 = stFt.tile([128, HT], F32, tag="sg2")
                nc.scalar.activation(sg2[:], acc[:, c, :], AF.Sigmoid,
                                     scale=s1[:], bias=s2[:])
                yt = stFt.tile([128, HT], F32, tag="yt")
                nc.vector.tensor_scalar(yt[:], acc[:, c, :],
                                        s1[:], s2[:], ALU.mult, ALU.add)
                nc.vector.tensor_mul(ysl[:, c, :], yt[:], sg2[:])
            with tc.tile_pool(name="cvp2", bufs=2, space="PSUM") as cps2:
                for f in range(4):
                    psw = cps2.tile([128, HT], F32, tag="psw")
                    for c in range(4):
                        nc.tensor.matmul(psw[:], w2[:, c, ts(f, 128)],
                                         ysl[:, c, :],
                                         start=(c == 0), stop=(c == 3))
                    nc.vector.scalar_tensor_tensor(
                        out=h3_sb[:, f, :], in0=psw[:], scalar=bp2[:, f:f + 1],
                        in1=h2w[:, f, 16:16 + HT], op0=ALU.add, op1=ALU.add)
                    nc.vector.tensor_copy(h3_bf[:, f, :],
                                          h3_sb[:, f, :].bitcast(F32))

        # ================= Stage G: FFN2 =================
        with tc.tile_pool(name="stG", bufs=1) as stG, \
             tc.tile_pool(name="stGt", bufs=2) as stGt:
            rb4 = stG.tile([128, HT], F32)
            nb4 = stG.tile([128, HT], F32)
            emit_stats(h3_sb[:], 4, HT, [(0, HT)], rb4, nb4, 0, stGt,
                       _r(ones_c[:]))
            emit_ffn(h3_bf[:], h3_sb[:], rb4, nb4, h4_sb[:], None,
                     wg1_d, sg1_d, bg1_d, wg2_d, bg2_d, "f2")

        # ================= Stage H: LN5 + output =================
        with tc.tile_pool(name="stH", bufs=1) as stH, \
             tc.tile_pool(name="stHt", bufs=2) as stHt:
            g5_sb = stH.tile([128, 4], F32)
            nc.sync.dma_start(out=g5_sb[:], in_=g5_d[:])
            b5_sb = stH.tile([128, 4], F32)
            nc.sync.dma_start(out=b5_sb[:], in_=b5_d[:])
            rb5 = stH.tile([128, HT], F32)
            nb5 = stH.tile([128, HT], F32)
            emit_stats(h4_sb[:], 4, HT, [(0, HT)], rb5, nb5, 0, stHt,
                       _r(ones_c[:]))
            for c in range(4):
                u = stHt.tile([128, HT], F32, tag="h_u")
                nc.vector.tensor_mul(u[:], h4_sb[:, c, :], rb5[:])
                w_ = stHt.tile([128, HT], F32, tag="h_w")
                nc.vector.tensor_add(w_[:], u[:], nb5[:])
                xn5 = stHt.tile([128, HT], F32, tag="h_o")
                nc.vector.tensor_scalar(xn5[:], w_[:],
                                        g5_sb[:, c:c + 1], b5_sb[:, c:c + 1],
                                        ALU.mult, ALU.add)
                nc.sync.dma_start(out=out_d[c], in_=xn5[:])
    return nc


_CACHE = {}


def build_nc():
    if "nc" not in _CACHE:
        nc = bacc.Bacc("TRN2", target_bir_lowering=False, debug=False,
                       num_devices=NCORES)
        _emit(nc)
        nc.compile()
        _CACHE["nc"] = nc
    return _CACHE["nc"]


def _chunk_cf(a2d):
    """[Dany, W] -> [Dany//128, 128, W] chunk-major channels-first."""
    d, w = a2d.shape
    return np.ascontiguousarray(a2d.reshape(d // 128, 128, w), dtype=np.float32)


def to_bf16(a):
    import ml_dtypes
    return np.asarray(a, dtype=np.float32).astype(ml_dtypes.bfloat16)


def round_bf16(a):
    import ml_dtypes
    return np.asarray(a, dtype=np.float32).astype(
        ml_dtypes.bfloat16).astype(np.float32)


def _pcol(vec):
    """[Dout] per-channel vector -> [128, Dout//128] (partition, chunk)."""
    n = vec.shape[0]
    return np.ascontiguousarray(vec.reshape(n // 128, 128).T, dtype=np.float32)


def make_in_maps(inputs):
    inputs = {k: np.asarray(v, dtype=np.float32) for k, v in inputs.items()}
    x = inputs["x"]; pos_emb = inputs["pos_emb"]
    ln1_g, ln1_b = inputs["ln1_g"], inputs["ln1_b"]
    ln2_g, ln2_b = inputs["ln2_g"], inputs["ln2_b"]
    ln3_g, ln3_b = inputs["ln3_g"], inputs["ln3_b"]
    ln4_g, ln4_b = inputs["ln4_g"], inputs["ln4_b"]
    ln5_g, ln5_b = inputs["ln5_g"], inputs["ln5_b"]

    # FFN1/FFN2: W' = diag(g) W (bf16), b' = b + ln_b @ W, S = colsum(W')
    w1f = round_bf16(ln1_g[:, None] * inputs["ff1_w1"])
    b1f = inputs["ff1_b1"] + ln1_b @ inputs["ff1_w1"]
    s1f = w1f.sum(axis=0)
    w2f = round_bf16(0.5 * inputs["ff1_w2"]); b2f = 0.5 * inputs["ff1_b2"]
    wg1f = round_bf16(ln4_g[:, None] * inputs["ff2_w1"])
    bg1f = inputs["ff2_b1"] + ln4_b @ inputs["ff2_w1"]
    sg1f = wg1f.sum(axis=0)
    wg2f = round_bf16(0.5 * inputs["ff2_w2"]); bg2f = 0.5 * inputs["ff2_b2"]

    s = DK ** -0.5
    pos_u_f = inputs["pos_u"].reshape(D); pos_v_f = inputs["pos_v"].reshape(D)
    wqf = round_bf16(s * (ln2_g[:, None] * inputs["wq"]))
    bqf = s * (inputs["bq"] + ln2_b @ inputs["wq"] + pos_u_f)
    sqf = wqf.sum(axis=0)
    dqvf = s * (pos_v_f - pos_u_f)
    wkf = round_bf16(ln2_g[:, None] * inputs["wk"])
    bkf = inputs["bk"] + ln2_b @ inputs["wk"]
    skf = wkf.sum(axis=0)
    wvf = round_bf16(ln2_g[:, None] * inputs["wv"])
    bvf = inputs["bv"] + ln2_b @ inputs["wv"]
    svf = wvf.sum(axis=0)
    posT = np.zeros((D, 2048), dtype=np.float32)
    posT[:, :PB] = pos_emb[0].T

    pw1f = round_bf16((inputs["pw1_w"] * ln3_g[None, :]).T)    # [512, 1024]
    bpw1f = inputs["pw1_b"] + inputs["pw1_w"] @ ln3_b          # [1024]
    spwf = pw1f.sum(axis=0)
    dwwf = inputs["dw_w"][:, 0, :]                             # [512, 31]
    pw2f = round_bf16(inputs["pw2_w"].T)                       # [512, 512]

    base = {
        "wf1": to_bf16(_chunk_cf(w1f)), "sf1": _pcol(s1f), "bf1": _pcol(b1f),
        "wf2": to_bf16(_chunk_cf(w2f)), "bf2": _pcol(b2f),
        "wg1": to_bf16(_chunk_cf(wg1f)), "sg1": _pcol(sg1f),
        "bg1": _pcol(bg1f),
        "wg2": to_bf16(_chunk_cf(wg2f)), "bg2": _pcol(bg2f),
        "posT": to_bf16(_chunk_cf(posT)),
        "pw1": to_bf16(_chunk_cf(pw1f)), "spw": _pcol(spwf),
        "bpw1": _pcol(bpw1f),
        "dwv": np.ascontiguousarray(
            dwwf.reshape(4, 128, KCONV).transpose(1, 0, 2),
            dtype=np.float32),
        "bng": _pcol(inputs["bn_g"]), "bnb": _pcol(inputs["bn_b"]),
        "pw2": to_bf16(_chunk_cf(pw2f)), "bpw2": _pcol(inputs["pw2_b"]),
        "bo": _pcol(inputs["bo"]),
        "g5": _pcol(ln5_g), "b5": _pcol(ln5_b),
        "onc": np.ones((128, 1), dtype=np.float32),
        "onc_bf": to_bf16(np.ones((128, 1))),
        "idn": np.eye(128, dtype=np.float32),
    }

    in_maps = []
    for c in range(NCORES):
        b, scr = c // 2, c % 2
        cols = slice(256 * scr, 256 * scr + 256)
        m = dict(base)
        xb = x[b, scr * HT:(scr + 1) * HT, :].T               # [512, 512]
        m["x_loc"] = _chunk_cf(xb)
        m["x_bf"] = to_bf16(_chunk_cf(xb))
        m["wq"] = to_bf16(_chunk_cf(wqf[:, cols]))
        m["sq"] = _pcol(sqf[cols]); m["bq"] = _pcol(bqf[cols])
        m["dqv"] = _pcol(bqf[cols] + dqvf[cols])   # bqv = bq + dqv
        m["wk"] = to_bf16(_chunk_cf(wkf[:, cols]))
        m["sk"] = _pcol(skf[cols]); m["bk"] = _pcol(bkf[cols])
        m["wv"] = to_bf16(_chunk_cf(wvf[:, cols]))
        m["svrow"] = np.ascontiguousarray(svf[cols].reshape(1, 256),
                                          dtype=np.float32)
        m["bvrow"] = np.ascontiguousarray(bvf[cols].reshape(1, 256),
                                          dtype=np.float32)
        m["wp"] = to_bf16(_chunk_cf(inputs["wp"][:, cols]))
        wo_rows = inputs["wo"][cols, :]                       # [256, 512]
        m["wo"] = to_bf16(np.ascontiguousarray(wo_rows.reshape(4, 64, D)))
        cmask = np.ones((1, WIN), dtype=np.float32)
        if scr == 0:
            cmask[0, :16] = 0.0
        else:
            cmask[0, WIN - 16:] = 0.0
        m["cmask"] = cmask
        in_maps.append(m)
    return in_maps


def assemble_out(results):
    out = np.empty((B, T, D), dtype=np.float32)
    for c in range(NCORES):
        b, scr = c // 2, c % 2
        ol = np.asarray(results[c]["out_loc"])                # [4, 128, 512]
        out[b, scr * HT:(scr + 1) * HT, :] = ol.reshape(D, HT).T
    return out


def kernel(**inputs):
    in_maps = make_in_maps(inputs)
    nc = build_nc()
    res = run_bass_kernel_spmd(nc, in_maps, list(range(NCORES)))
    return assemble_out(res.results)


# revision 51
# speedup vs baseline: 1.1066x; 1.0204x over previous
"""Conformer layer on 8 Trainium2 NeuronCores (v2).

Sharding: core c handles batch b=c//2. Within a batch pair:
 - token-parallel (halves of T=1024) for FFN1/conv/FFN2/LN stages,
 - head-parallel (4 heads each) for attention.

v2 changes vs baseline:
 - LayerNorm fold: matmuls consume raw activations; per-token scale/shift is
   applied as a post-matmul fixup z = psy*rec + colsum(W')*(-m*rec) + b', so
   the PE never waits for LN statistics (keeps the HAM clock warm).
 - bf16 operands for attention (q/k/v/p/probs/o_h) with moving-dim-1024
   matmuls, bf16 FFN/pw/wo weights (same PE rate, half the DMA).
 - AllGather carries bf16 h and overlaps with p-projection + local-half QKV;
   a warmup collective at kernel start absorbs the first-cc latency.
 - Attention ReduceScatter in bf16.
 - Depthwise-conv diagonal matrices built on-chip (saves an 8MB DMA).
 - Plain loads on HWDGE (nc.sync), freeing GpSimd for casts/broadcasts.
"""

import numpy as np

import concourse.bass as bass
import concourse.mybir as mybir
import concourse.tile as tile
from concourse import bacc
from concourse.bass import ds, ts
from concourse.bass_utils import run_bass_kernel_spmd
from contextlib import ExitStack

F32 = mybir.dt.float32
F32R = mybir.dt.float32r
BF16 = mybir.dt.bfloat16
AF = mybir.ActivationFunctionType
ALU = mybir.AluOpType

D, DFF, H, DK, KCONV = 512, 2048, 8, 64, 31
B, T = 4, 1024
EPS = 1e-5
HT = 512            # tokens per core
WIN = 544           # conv window: 16 + 512 + 16
PB = 2047
BAND = 1152         # bd band width per q-chunk
BST = 1160          # bd dram row stride (elements)
NCORES = 8

PAIRS = [[0, 1], [2, 3], [4, 5], [6, 7]]
ALLG = [[0, 1, 2, 3, 4, 5, 6, 7]]


def _r(ap):
    return ap.bitcast(F32R)


def _emit(nc):
    def inp(name, shape, dt=F32):
        return nc.dram_tensor(name, list(shape), dt, kind="ExternalInput")

    x_d = inp("x_loc", (4, 128, HT), F32R)
    xbf_d = inp("x_bf", (4, 128, HT), BF16)
    wf1_d = inp("wf1", (4, 128, DFF), BF16)
    sf1_d = inp("sf1", (128, 16)); bf1_d = inp("bf1", (128, 16))
    wf2_d = inp("wf2", (16, 128, D), BF16); bf2_d = inp("bf2", (128, 4))
    wq_d = inp("wq", (4, 128, 256), BF16)
    sq_d = inp("sq", (128, 2)); bq_d = inp("bq", (128, 2))
    dqv_d = inp("dqv", (128, 2))
    wk_d = inp("wk", (4, 128, 256), BF16)
    sk_d = inp("sk", (128, 2)); bk_d = inp("bk", (128, 2))
    wv_d = inp("wv", (4, 128, 256), BF16)
    sv_d = inp("svrow", (1, 256)); bv_d = inp("bvrow", (1, 256))
    wp_d = inp("wp", (4, 128, 256), BF16)
    wo_d = inp("wo", (4, 64, D), BF16); bo_d = inp("bo", (128, 4))
    posT_d = inp("posT", (4, 128, 2048), BF16)
    pw1_d = inp("pw1", (4, 128, 1024), BF16)
    spw_d = inp("spw", (128, 8)); bpw1_d = inp("bpw1", (128, 8))
    dwv_d = inp("dwv", (128, 4, KCONV))
    bng_d = inp("bng", (128, 4)); bnb_d = inp("bnb", (128, 4))
    pw2_d = inp("pw2", (4, 128, D), BF16); bpw2_d = inp("bpw2", (128, 4))
    cmask_d = inp("cmask", (1, WIN))
    wg1_d = inp("wg1", (4, 128, DFF), BF16)
    sg1_d = inp("sg1", (128, 16)); bg1_d = inp("bg1", (128, 16))
    wg2_d = inp("wg2", (16, 128, D), BF16); bg2_d = inp("bg2", (128, 4))
    g5_d = inp("g5", (128, 4)); b5_d = inp("b5", (128, 4))
    onc_d = inp("onc", (128, 1), F32R)
    oncb_d = inp("onc_bf", (128, 1), BF16)
    idn_d = inp("idn", (128, 128))

    out_d = nc.dram_tensor("out_loc", [4, 128, HT], F32, kind="ExternalOutput")

    cc_w_in = nc.dram_tensor("cc_w_in", [1, 64], F32)
    cc_w_out = nc.dram_tensor("cc_w_out", [2, 64], F32)
    cc_h_in = nc.dram_tensor("cc_h_in", [4, 128, HT], F32)
    cc_h_out = nc.dram_tensor("cc_h_out", [2, 4, 128, HT], F32)
    cc_a_in = nc.dram_tensor("cc_a_in", [2, 4, 128, WIN], F32)
    cc_a_out = nc.dram_tensor("cc_a_out", [4, 128, WIN], F32)
    cc_bn_in = nc.dram_tensor("cc_bn_in", [128, 8], F32)
    cc_bn_out = nc.dram_tensor("cc_bn_out", [128, 8], F32)
    bd_d = [nc.dram_tensor(f"bd_{i}", [128 * BST], BF16) for i in range(32)]

    uid = [0]

    with tile.TileContext(nc) as tc, ExitStack() as ctx:
        const = ctx.enter_context(tc.tile_pool(name="const", bufs=1))
        # CC-stream warmup first: tiny 8-way AllReduce absorbs the bootstrap
        # barrier + cross-core startup skew while FFN1 runs.
        warm = const.tile([1, 64], F32)
        nc.vector.memset(warm[:], 0.0)
        nc.gpsimd.dma_start(out=cc_w_in[:], in_=warm[:])
        nc.gpsimd.collective_compute(
            "AllReduce", ALU.add, ins=[cc_w_in[:]], outs=[cc_w_out[0:1]],
            replica_groups=ALLG)

        ones_c = const.tile([128, 1], F32R)
        nc.sync.dma_start(out=ones_c[:], in_=onc_d[:])
        ones_cb = const.tile([128, 1], BF16)
        nc.sync.dma_start(out=ones_cb[:], in_=oncb_d[:])
        eps1 = const.tile([1, 1], F32); nc.vector.memset(eps1[:], EPS)
        epsP = const.tile([128, 1], F32); nc.vector.memset(epsP[:], EPS)
        identb = const.tile([128, 128], BF16)
        nc.gpsimd.dma_start(out=identb[:], in_=idn_d[:])
        def pacer(psp, sbp, rounds, tag):
            # PE<->DVE ping-pong that keeps the HAM activity window busy
            # across a collective wait. Each round ~1.5us of wall coverage
            # at ~0.3us of engine time. Values stay zero.
            ps = psp.tile([128, 256], F32, tag="pc" + tag)
            sb = sbp.tile([128, 256], BF16, tag="pcs" + tag)
            nc.vector.memset(sb[:], 0.0)
            for r in range(rounds):
                nc.tensor.matmul(ps[:], sb[0:128, 0:128], sb[:, 0:256],
                                 start=True, stop=True)
                nc.vector.tensor_copy(sb[:], ps[:])

        # per-engine copies of the core's token-half index (register values
        # are engine-local)
        pidv = nc.vector.partition_id()
        scv, scv2 = pidv % 2, (pidv + 1) % 2
        pida = nc.scalar.partition_id()
        sca, sca2 = pida % 2, (pida + 1) % 2
        pids = nc.sync.partition_id()
        scs, scs2 = pids % 2, (pids + 1) % 2

        act = ctx.enter_context(tc.tile_pool(name="act", bufs=1))
        hfe_sb = act.tile([128, 4, WIN], F32R)   # conv residual window
        h_sb = act.tile([128, 4, HT], F32R)      # post-FFN1 hidden (local)
        h_bf = act.tile([128, 4, HT], BF16)
        h3_sb = act.tile([128, 4, HT], F32R)     # post-conv hidden
        h3_bf = act.tile([128, 4, HT], BF16)
        h4_sb = act.tile([128, 4, HT], F32R)     # post-FFN2 hidden

        # ---------- LN statistics (fold form) ----------
        # Produces broadcast tiles RB = 1/std and NM = -mean/std per token.
        def emit_stats(x4, nchunk, W, blocks, rb_t, nb_t, col0, sbp, ones):
            uid[0] += 1
            with tc.tile_pool(name=f"lnps{uid[0]}", bufs=1,
                              space="PSUM") as lnps:
                x2 = sbp.tile([128, nchunk, W], F32R, tag="ln_sq")
                nc.scalar.square(x2[:], x4)
                for b0, bw in blocks:
                    pss = lnps.tile([1, bw], F32, tag="lns")
                    psq = lnps.tile([1, bw], F32, tag="lnq")
                    for c in range(nchunk):
                        nc.tensor.matmul(pss[:], ones,
                                         x4[:, c, b0:b0 + bw],
                                         start=(c == 0), stop=(c == nchunk - 1))
                    for c in range(nchunk):
                        nc.tensor.matmul(psq[:], _r(ones_c[:]),
                                         _r(x2[:, c, b0:b0 + bw]),
                                         start=(c == 0), stop=(c == nchunk - 1))
                    mng = sbp.tile([1, bw], F32, tag="ln_m")
                    nc.scalar.activation(mng[:], pss[:], AF.Copy,
                                         scale=-1.0 / D)
                    e2 = sbp.tile([1, bw], F32, tag="ln_e2")
                    nc.scalar.activation(e2[:], psq[:], AF.Copy, scale=1.0 / D)
                    var = sbp.tile([1, bw], F32, tag="ln_var")
                    nc.vector.tensor_mul(var[:], mng[:], mng[:])
                    nc.vector.tensor_sub(var[:], e2[:], var[:])
                    sd = sbp.tile([1, bw], F32, tag="ln_sd")
                    nc.scalar.activation(sd[:], var[:], AF.Sqrt, bias=eps1[:])
                    rec = sbp.tile([1, bw], F32, tag="ln_rs")
                    scr = sbp.tile([1, bw], F32, tag="ln_scr")
                    nc.vector.reciprocal_approx_accurate(rec[:], sd[:], scr[:])
                    nmr = sbp.tile([1, bw], F32, tag="ln_nm")
                    nc.vector.tensor_mul(nmr[:], mng[:], rec[:])
                    nc.gpsimd.partition_broadcast(
                        rb_t[:, col0 + b0:col0 + b0 + bw], rec[:])
                    nc.gpsimd.partition_broadcast(
                        nb_t[:, col0 + b0:col0 + b0 + bw], nmr[:])

        # ---------- FFN with LN fold ----------
        def emit_ffn(xbf, xres, rb_t, nb_t, out, outbf, w1dram, s1dram,
                     b1dram, w2dram, b2dram, pref):
            with tc.tile_pool(name=pref + "w", bufs=1) as wp_, \
                 tc.tile_pool(name=pref + "t", bufs=3) as tp, \
                 tc.tile_pool(name=pref + "ps", bufs=2, space="PSUM") as psp, \
                 tc.tile_pool(name=pref + "ph", bufs=1, space="PSUM") as php:
                w1 = wp_.tile([128, 4, DFF], BF16)
                for c in range(4):
                    nc.sync.dma_start(out=w1[:, c, :], in_=w1dram[c])
                w2 = wp_.tile([128, 16, D], BF16)
                for j in range(16):
                    nc.sync.dma_start(out=w2[:, j, :], in_=w2dram[j])
                s1 = wp_.tile([128, 16], F32)
                nc.sync.dma_start(out=s1[:], in_=s1dram[:])
                b1 = wp_.tile([128, 16], F32)
                nc.sync.dma_start(out=b1[:], in_=b1dram[:])
                b2 = wp_.tile([128, 4], F32)
                nc.sync.dma_start(out=b2[:], in_=b2dram[:])
                psh = php.tile([128, 4, HT], F32)
                for j in range(16):
                    psy = psp.tile([128, HT], F32, tag="psy")
                    for c in range(4):
                        nc.tensor.matmul(psy[:], w1[:, c, ts(j, 128)],
                                         xbf[:, c, :],
                                         start=(c == 0), stop=(c == 3))
                    u = tp.tile([128, HT], F32, tag="u")
                    nc.vector.tensor_mul(u[:], psy[:], rb_t[:])
                    z = tp.tile([128, HT], F32, tag="z")
                    nc.vector.scalar_tensor_tensor(
                        out=z[:], in0=nb_t[:], scalar=s1[:, j:j + 1],
                        in1=u[:], op0=ALU.mult, op1=ALU.add)
                    sig = tp.tile([128, HT], F32, tag="sig")
                    nc.scalar.activation(sig[:], z[:], AF.Sigmoid,
                                         bias=b1[:, j:j + 1])
                    y1 = tp.tile([128, HT], BF16, tag="y1")
                    nc.vector.scalar_tensor_tensor(
                        out=y1[:], in0=z[:], scalar=b1[:, j:j + 1],
                        in1=sig[:], op0=ALU.add, op1=ALU.mult)
                    for f in range(4):
                        nc.tensor.matmul(psh[:, f, :],
                                         w2[:, j, ts(f, 128)], y1[:],
                                         start=(j == 0), stop=(j == 15))
                for c in range(4):
                    nc.vector.scalar_tensor_tensor(
                        out=out[:, c, :], in0=psh[:, c, :],
                        scalar=b2[:, c:c + 1], in1=xres[:, c, :],
                        op0=ALU.add, op1=ALU.add)
                    if outbf is not None:
                        nc.vector.tensor_copy(outbf[:, c, :],
                                              out[:, c, :].bitcast(F32))

        # ================= Stage A: FFN1 on local tokens =================
        with tc.tile_pool(name="stA", bufs=1) as stA, \
             tc.tile_pool(name="stAt", bufs=2) as stAt:
            x_sb = stA.tile([128, 4, HT], F32R)
            for c in range(4):
                nc.gpsimd.dma_start(out=x_sb[:, c, :], in_=x_d[c])
            x_bf = stA.tile([128, 4, HT], BF16)
            for c in range(4):
                nc.sync.dma_start(out=x_bf[:, c, :], in_=xbf_d[c])
            rb1 = stA.tile([128, HT], F32)
            nb1 = stA.tile([128, HT], F32)
            emit_stats(x_sb[:], 4, HT, [(0, HT)], rb1, nb1, 0, stAt,
                       _r(ones_c[:]))
            emit_ffn(x_bf[:], x_sb[:], rb1, nb1, h_sb[:], h_bf[:],
                     wf1_d, sf1_d, bf1_d, wf2_d, bf2_d, "f1")
            # conv residual window: local half occupies [16, 528)
            for c in range(4):
                nc.vector.tensor_copy(hfe_sb[:, c, 16:16 + HT].bitcast(F32),
                                      h_sb[:, c, :].bitcast(F32))
            for c in range(4):
                nc.sync.dma_start(out=cc_h_in[c],
                                  in_=h_sb[:, c, :].bitcast(F32))

        nc.gpsimd.collective_compute(
            "AllGather", ALU.bypass, ins=[cc_h_in[:]], outs=[cc_h_out[:]],
            replica_groups=PAIRS)

        # ============ attention scope ============
        with tc.tile_pool(name="attp", bufs=1) as attp:
            woff = sca * HT                # local half start (ACT offsets)
            roff = sca2 * HT               # remote half start

            q_sb = attp.tile([128, 2, T], BF16)
            qv_sb = attp.tile([128, 2, T], BF16)
            k_sb = attp.tile([128, 2, T], BF16)
            v65 = attp.tile([128, 8, 4, 65], BF16)
            nc.vector.memset(v65[:, :, :, 64:65], 1.0)
            p_sb = attp.tile([128, 2, 2048], BF16)
            # rb2/nb2 and recc/nmc are stored local-half-first (static
            # writes); the eviction target offsets stay symbolic.
            rb2 = attp.tile([128, T], F32)
            nb2 = attp.tile([128, T], F32)
            recc = attp.tile([128, 8], F32)
            nmc = attp.tile([128, 8], F32)

            # ---- p-projection (overlaps AllGather) ----
            with tc.tile_pool(name="ppp", bufs=2, space="PSUM") as ppp, \
                 tc.tile_pool(name="atw", bufs=1) as atw:
                wp_sb = atw.tile([128, 4, 256], BF16, tag="wp")
                for c in range(4):
                    nc.sync.dma_start(out=wp_sb[:, c, :], in_=wp_d[c])
                pos_sb = atw.tile([128, 4, 2048], BF16, tag="pos")
                for c in range(4):
                    nc.sync.dma_start(out=pos_sb[:, c, :], in_=posT_d[c])
                for m in range(2):
                    for pc in range(4):
                        psp_t = ppp.tile([128, 512], F32, tag="psp")
                        for c in range(4):
                            nc.tensor.matmul(
                                psp_t[:], wp_sb[:, c, ts(m, 128)],
                                pos_sb[:, c, ts(pc, 512)],
                                start=(c == 0), stop=(c == 3))
                        nc.scalar.activation(p_sb[:, m, ts(pc, 512)],
                                             psp_t[:], AF.Copy)

            # ---- local-half LN2 stats + qkv (overlap AllGather) ----
            with tc.tile_pool(name="stC", bufs=1) as stC, \
                 tc.tile_pool(name="stCt", bufs=2) as stCt, \
                 tc.tile_pool(name="qkp", bufs=2, space="PSUM") as qkps:

                def emit_stats2(x4, ones, rcol):
                    # LN2 stats for one half; writes rb2/nb2[:, rcol:rcol+HT]
                    # and recc/nmc[:, rcol//128 : +4] (static columns).
                    uid[0] += 1
                    cc = rcol // 128
                    with tc.tile_pool(name=f"lnps{uid[0]}", bufs=1,
                                      space="PSUM") as lnps:
                        x2 = stCt.tile([128, 4, HT], F32R, tag="ln_sq")
                        nc.scalar.square(x2[:], x4)
                        pss = lnps.tile([1, HT], F32, tag="lns")
                        psq = lnps.tile([1, HT], F32, tag="lnq")
                        for c in range(4):
                            nc.tensor.matmul(pss[:], ones, x4[:, c, :],
                                             start=(c == 0), stop=(c == 3))
                        for c in range(4):
                            nc.tensor.matmul(psq[:], _r(ones_c[:]),
                                             _r(x2[:, c, :]),
                                             start=(c == 0), stop=(c == 3))
                        mng = stCt.tile([1, HT], F32, tag="ln_m")
                        nc.scalar.activation(mng[:], pss[:], AF.Copy,
                                             scale=-1.0 / D)
                        e2 = stCt.tile([1, HT], F32, tag="ln_e2")
                        nc.scalar.activation(e2[:], psq[:], AF.Copy,
                                             scale=1.0 / D)
                        var = stCt.tile([1, HT], F32, tag="ln_var")
                        nc.vector.tensor_mul(var[:], mng[:], mng[:])
                        nc.vector.tensor_sub(var[:], e2[:], var[:])
                        sd = stCt.tile([1, HT], F32, tag="ln_sd")
                        nc.scalar.activation(sd[:], var[:], AF.Sqrt,
                                             bias=eps1[:])
                        rec2 = stCt.tile([1, HT], F32, tag="ln_rec")
                        scr = stCt.tile([1, HT], F32, tag="ln_scr")
                        nc.vector.reciprocal_approx_accurate(rec2[:], sd[:],
                                                             scr[:])
                        nm2 = stCt.tile([1, HT], F32, tag="ln_nm")
                        nc.vector.tensor_mul(nm2[:], mng[:], rec2[:])
                        nc.gpsimd.partition_broadcast(
                            rb2[:, rcol:rcol + HT], rec2[:])
                        nc.gpsimd.partition_broadcast(
                            nb2[:, rcol:rcol + HT], nm2[:])
                        for b in range(4):
                            nc.gpsimd.dma_start(
                                out=recc[:, cc + b:cc + b + 1],
                                in_=rec2[:, ts(b, 128)])
                            nc.gpsimd.dma_start(
                                out=nmc[:, cc + b:cc + b + 1],
                                in_=nm2[:, ts(b, 128)])

                wq_sb = stC.tile([128, 4, 256], BF16)
                wk_sb = stC.tile([128, 4, 256], BF16)
                wv_sb = stC.tile([128, 4, 256], BF16)
                for c in range(4):
                    nc.sync.dma_start(out=wq_sb[:, c, :], in_=wq_d[c])
                    nc.sync.dma_start(out=wk_sb[:, c, :], in_=wk_d[c])
                    nc.sync.dma_start(out=wv_sb[:, c, :], in_=wv_d[c])
                sq_sb = stC.tile([128, 2], F32)
                nc.sync.dma_start(out=sq_sb[:], in_=sq_d[:])
                bq_sb = stC.tile([128, 2], F32)
                nc.sync.dma_start(out=bq_sb[:], in_=bq_d[:])
                sk_sb = stC.tile([128, 2], F32)
                nc.sync.dma_start(out=sk_sb[:], in_=sk_d[:])
                bk_sb = stC.tile([128, 2], F32)
                nc.sync.dma_start(out=bk_sb[:], in_=bk_d[:])
                bqv_sb = stC.tile([128, 2], F32)
                nc.sync.dma_start(out=bqv_sb[:], in_=dqv_d[:])
                sv_row = stC.tile([1, 256], F32)
                nc.sync.dma_start(out=sv_row[:], in_=sv_d[:])
                svb = stC.tile([128, 256], F32)
                nc.gpsimd.partition_broadcast(svb[:], sv_row[:])
                bv_row = stC.tile([1, 256], F32)
                nc.sync.dma_start(out=bv_row[:], in_=bv_d[:])
                bvb = stC.tile([128, 256], F32)
                nc.gpsimd.partition_broadcast(bvb[:], bv_row[:])

                def emit_qk(xin, rcol, off):
                    # rcol: static column base in rb2/nb2 (local-first);
                    # off: symbolic global token offset for the outputs.
                    for m in range(2):
                        for w_sb, s_w, b_w, dst in (
                                (wq_sb, sq_sb, bq_sb, q_sb),
                                (wk_sb, sk_sb, bk_sb, k_sb)):
                            ps = qkps.tile([128, HT], F32, tag="psqk")
                            for c in range(4):
                                nc.tensor.matmul(
                                    ps[:], w_sb[:, c, ts(m, 128)],
                                    xin[:, c, :],
                                    start=(c == 0), stop=(c == 3))
                            u = stCt.tile([128, HT], F32, tag="qk_u")
                            nc.vector.tensor_mul(u[:], ps[:],
                                                 rb2[:, rcol:rcol + HT])
                            w_ = stCt.tile([128, HT], F32, tag="qk_w")
                            nc.vector.scalar_tensor_tensor(
                                out=w_[:], in0=nb2[:, rcol:rcol + HT],
                                scalar=s_w[:, m:m + 1], in1=u[:],
                                op0=ALU.mult, op1=ALU.add)
                            nc.scalar.activation(
                                dst[:, m, ds(off, HT)], w_[:], AF.Identity,
                                bias=b_w[:, m:m + 1])
                            if dst is q_sb:
                                nc.scalar.activation(
                                    qv_sb[:, m, ds(off, HT)], w_[:],
                                    AF.Identity, bias=bqv_sb[:, m:m + 1])

                def emit_v(xin, vbase, rbase):
                    # vbase: symbolic global chunk base; rbase: static
                    # column base into recc/nmc.
                    for tq in range(4):
                        psv = qkps.tile([128, 256], F32, tag="psv")
                        for c in range(4):
                            nc.tensor.matmul(
                                psv[:], xin[:, c, ts(tq, 128)],
                                wv_sb[:, c, :],
                                start=(c == 0), stop=(c == 3))
                        u = stCt.tile([128, 256], F32, tag="v_u")
                        nc.vector.tensor_scalar_mul(
                            u[:], psv[:], recc[:, rbase + tq:rbase + tq + 1])
                        w_ = stCt.tile([128, 256], F32, tag="v_w")
                        nc.vector.scalar_tensor_tensor(
                            out=w_[:], in0=svb[:],
                            scalar=nmc[:, rbase + tq:rbase + tq + 1],
                            in1=u[:], op0=ALU.mult, op1=ALU.add)
                        vt = stCt.tile([128, 256], BF16, tag="v_t")
                        nc.vector.tensor_add(vt[:], w_[:], bvb[:])
                        nc.vector.tensor_copy(
                            v65[:, ds(vbase + tq, 1), :, 0:64]
                            .rearrange("p o h d -> p (o h) d"),
                            vt[:].rearrange("p (h d) -> p h d", h=4))

                emit_stats2(h_sb[:], _r(ones_c[:]), 0)
                emit_qk(h_bf[:], 0, woff)
                emit_v(h_bf[:], scv * 4, 0)

                # ---- gather lands: remote half ----
                h_rem = stC.tile([128, 4, HT], F32R)
                for c in range(4):
                    nc.sync.dma_start(
                        out=h_rem[:, c, :].bitcast(F32),
                        in_=cc_h_out[:].rearrange("s c p t -> (s c) p t")
                        [ds(scs2 * 4 + c, 1)]
                        .rearrange("o p t -> (o p) t"))
                h_rem_bf = stC.tile([128, 4, HT], BF16)
                for c in range(4):
                    nc.vector.tensor_copy(h_rem_bf[:, c, :],
                                          h_rem[:, c, :].bitcast(F32))
                # conv halo: zero edges then copy 16 remote tokens
                nc.vector.memset(hfe_sb[:, :, 0:16].bitcast(F32), 0.0)
                nc.vector.memset(hfe_sb[:, :, 16 + HT:].bitcast(F32), 0.0)
                for c in range(4):
                    nc.vector.tensor_copy(
                        hfe_sb[:, c, ds(scv2 * (16 + HT), 16)].bitcast(F32),
                        h_rem[:, c, ds(scv * (HT - 16), 16)].bitcast(F32))

                emit_stats2(h_rem[:], _r(ones_c[:]), HT)
                emit_qk(h_rem_bf[:], HT, roff)
                emit_v(h_rem_bf[:], scv2 * 4, 4)

            # ---- bd banded matmuls for all heads ----
            with tc.tile_pool(name="bds", bufs=2) as bdp, \
                 tc.tile_pool(name="psb", bufs=2, space="PSUM") as psb:
                for h in range(4):
                    hc, hr = h // 2, 64 * (h % 2)
                    for qc in range(8):
                        base = max(0, 895 - 128 * qc)
                        bdw = bdp.tile([128, BAND], BF16, tag="bdw")
                        for pi in range(3):
                            psB = psb.tile([128, 384], F32, tag="psB")
                            nc.tensor.matmul(
                                psB[:],
                                qv_sb[hr:hr + 64, hc, ts(qc, 128)],
                                p_sb[hr:hr + 64, hc,
                                     base + pi * 384: base + (pi + 1) * 384],
                                start=True, stop=True)
                            nc.vector.tensor_copy(
                                bdw[:, pi * 384:(pi + 1) * 384], psB[:])
                        nc.sync.dma_start(
                            out=bd_d[h * 8 + qc][:].rearrange(
                                "(p w) -> p w", p=128)[:, 0:BAND],
                            in_=bdw[:])

            # ---- attention heads: scores + softmax + AV ----
            o_h = [attp.tile([64, T], BF16, name=f"o_h{_h}", tag=f"o_h{_h}")
                   for _h in range(4)]
            with tc.tile_pool(name="bdsh", bufs=1) as shp, \
                 tc.tile_pool(name="atp", bufs=3) as atp, \
                 tc.tile_pool(name="atn", bufs=1) as atn, \
                 tc.tile_pool(name="pss", bufs=2, space="PSUM") as pss_p, \
                 tc.tile_pool(name="psav", bufs=2, space="PSUM") as psav:
                for h in range(4):
                    hc, hr = h // 2, 64 * (h % 2)
                    # XBAR transpose during the shifted read:
                    # bdshT[p, kc, q] = bd[q, kc*128+p]  (k-major layout)
                    bdshT = shp.tile([128, 8, T], BF16, tag=f"sh{h % 2}")
                    for qc in range(8):
                        c0 = 127 if qc == 7 else 128
                        src = bass.AP(tensor=bd_d[h * 8 + qc], offset=c0,
                                      ap=[[BST - 1, 128], [1, T]])
                        nc.sync.dma_start(
                            out=bdshT[:, :, ts(qc, 128)], in_=src,
                            transpose=True)
                    psA = psav.tile([65, T], F32, tag="psAV")
                    for kc in range(8):
                        psS = pss_p.tile([128, T], F32, tag="psS")
                        for th in range(2):
                            nc.tensor.matmul(
                                psS[:, th * 512:(th + 1) * 512],
                                k_sb[hr:hr + 64, hc, ts(kc, 128)],
                                q_sb[hr:hr + 64, hc,
                                     th * 512:(th + 1) * 512],
                                start=True, stop=False, skip_group_check=True)
                        for th in range(2):
                            nc.tensor.matmul(
                                psS[:, th * 512:(th + 1) * 512],
                                identb[:],
                                bdshT[:, kc, th * 512:(th + 1) * 512],
                                start=False, stop=True,
                                skip_group_check=True)
                        probs = atp.tile([128, T], BF16, tag="probs")
                        nc.scalar.activation(probs[:], psS[:], AF.Exp)
                        for th in range(2):
                            nc.tensor.matmul(
                                psA[:, th * 512:(th + 1) * 512],
                                v65[:, kc, h, :],
                                probs[:, th * 512:(th + 1) * 512],
                                start=(kc == 0), stop=(kc == 7),
                                skip_group_check=True)
                    s65 = atn.tile([65, T], F32, tag="s65")
                    nc.vector.tensor_copy(s65[64:65, :], psA[64:65, :])
                    row = atn.tile([1, T], F32, tag="row")
                    nc.gpsimd.dma_start(out=row[:], in_=s65[64:65, :])
                    rec = atn.tile([1, T], F32, tag="rec")
                    scr = atn.tile([1, T], F32, tag="scr")
                    nc.vector.reciprocal_approx_accurate(rec[:], row[:],
                                                         scr[:])
                    rb = atn.tile([64, T], F32, tag="rb")
                    nc.gpsimd.partition_broadcast(rb[:], rec[:])
                    nc.vector.tensor_mul(o_h[h][:], psA[0:64, :], rb[:])

            # ---- out-projection + pair ReduceScatter (bf16) ----
            with tc.tile_pool(name="pso", bufs=2, space="PSUM") as psop, \
                 tc.tile_pool(name="aot", bufs=2) as aot:
                wo_sb = aot.tile([64, 4, D], BF16, tag="wo", bufs=1)
                for hh in range(4):
                    nc.sync.dma_start(out=wo_sb[:, hh, :], in_=wo_d[hh])
                for f in range(4):
                    pso = psop.tile([128, T], F32, tag="pso")
                    for th in range(2):
                        for hh in range(4):
                            nc.tensor.matmul(
                                pso[:, th * 512:(th + 1) * 512],
                                wo_sb[:, hh, ts(f, 128)],
                                o_h[hh][:, th * 512:(th + 1) * 512],
                                start=(hh == 0), stop=(hh == 3),
                                skip_group_check=True)
                    ao = aot.tile([128, 2, WIN], F32, tag="ao")
                    nc.vector.memset(ao[:, 0, 0:16], 0.0)
                    nc.vector.memset(ao[:, 1, WIN - 16:WIN], 0.0)
                    nc.vector.tensor_copy(ao[:, 0, 16:WIN], pso[:, 0:528])
                    nc.vector.tensor_copy(ao[:, 1, 0:528], pso[:, 496:T])
                    nc.sync.dma_start(out=cc_a_in[0, f], in_=ao[:, 0, :])
                    nc.sync.dma_start(out=cc_a_in[1, f], in_=ao[:, 1, :])

        nc.gpsimd.collective_compute(
            "ReduceScatter", ALU.add, ins=[cc_a_in[:]], outs=[cc_a_out[:]],
            replica_groups=PAIRS)

        # ================= Stage F: conv module =================
        with tc.tile_pool(name="stF", bufs=1) as stF, \
             tc.tile_pool(name="stFt", bufs=2) as stFt:
            # on-chip depthwise diag build (overlaps ReduceScatter)
            dwv = stF.tile([128, 4, KCONV], F32)
            nc.sync.dma_start(out=dwv[:], in_=dwv_d[:])
            dg = stF.tile([128, 4, KCONV, 128], BF16)
            for c in range(4):
                for j in range(KCONV):
                    nc.vector.tensor_scalar(
                        dg[:, c, j, :], identb[:],
                        dwv[:, c, j:j + 1], 0.0, ALU.mult, ALU.add)
            w1 = stF.tile([128, 4, 1024], BF16)
            for c in range(4):
                nc.sync.dma_start(out=w1[:, c, :], in_=pw1_d[c])
            spw = stF.tile([128, 8], F32)
            nc.sync.dma_start(out=spw[:], in_=spw_d[:])
            bp1 = stF.tile([128, 8], F32)
            nc.sync.dma_start(out=bp1[:], in_=bpw1_d[:])
            w2 = stF.tile([128, 4, D], BF16)
            for c in range(4):
                nc.sync.dma_start(out=w2[:, c, :], in_=pw2_d[c])
            bp2 = stF.tile([128, 4], F32)
            nc.sync.dma_start(out=bp2[:], in_=bpw2_d[:])
            bo_sb = stF.tile([128, 4], F32)
            nc.sync.dma_start(out=bo_sb[:], in_=bo_d[:])
            cm = stF.tile([1, WIN], F32)
            nc.sync.dma_start(out=cm[:], in_=cmask_d[:])
            cmb = stF.tile([128, WIN], F32)
            nc.gpsimd.partition_broadcast(cmb[:], cm[:])

            h2w = stF.tile([128, 4, WIN], F32R)
            for c in range(4):
                at = stFt.tile([128, WIN], F32, tag="at")
                nc.sync.dma_start(out=at[:], in_=cc_a_out[c])
                nc.vector.scalar_tensor_tensor(
                    out=h2w[:, c, :], in0=at[:],
                    scalar=bo_sb[:, c:c + 1],
                    in1=hfe_sb[:, c, :].bitcast(F32),
                    op0=ALU.add, op1=ALU.add)
            h2w_bf = stF.tile([128, 4, WIN], BF16)
            for c in range(4):
                nc.vector.tensor_copy(h2w_bf[:, c, :],
                                      h2w[:, c, :].bitcast(F32))

            rb3 = stF.tile([128, WIN], F32)
            nb3 = stF.tile([128, WIN], F32)
            emit_stats(h2w[:], 4, WIN, [(0, 272), (272, 272)], rb3, nb3, 0,
                       stFt, _r(ones_c[:]))

            glu = stF.tile([128, 4, WIN], BF16)
            with tc.tile_pool(name="cvp1", bufs=1, space="PSUM") as cps:
                for m in range(4):
                    psa = cps.tile([128, 2, 512], F32, tag="psa")
                    psg = cps.tile([128, 2, 512], F32, tag="psg")
                    for half in range(2):
                        sl = slice(half * 272, (half + 1) * 272)
                        for c in range(4):
                            nc.tensor.matmul(psa[:, half, 0:272],
                                             w1[:, c, ts(m, 128)],
                                             h2w_bf[:, c, sl],
                                             start=(c == 0), stop=(c == 3),
                                             skip_group_check=True)
                        for c in range(4):
                            nc.tensor.matmul(psg[:, half, 0:272],
                                             w1[:, c, 512 + m * 128:
                                                 512 + (m + 1) * 128],
                                             h2w_bf[:, c, sl],
                                             start=(c == 0), stop=(c == 3),
                                             skip_group_check=True)
                    for half in range(2):
                        sl = slice(half * 272, (half + 1) * 272)
                        ua = stFt.tile([128, 272], F32, tag="cv_ua")
                        nc.vector.tensor_mul(ua[:], psa[:, half, 0:272],
                                             rb3[:, sl])
                        za = stFt.tile([128, 272], F32, tag="cv_za")
                        nc.vector.scalar_tensor_tensor(
                            out=za[:], in0=nb3[:, sl],
                            scalar=spw[:, m:m + 1], in1=ua[:],
                            op0=ALU.mult, op1=ALU.add)
                        ug = stFt.tile([128, 272], F32, tag="cv_ug")
                        nc.vector.tensor_mul(ug[:], psg[:, half, 0:272],
                                             rb3[:, sl])
                        zg = stFt.tile([128, 272], F32, tag="cv_zg")
                        nc.vector.scalar_tensor_tensor(
                            out=zg[:], in0=nb3[:, sl],
                            scalar=spw[:, 4 + m:5 + m], in1=ug[:],
                            op0=ALU.mult, op1=ALU.add)
                        sg = stFt.tile([128, 272], F32, tag="cv_sg")
                        nc.scalar.activation(sg[:], zg[:], AF.Sigmoid,
                                             bias=bp1[:, 4 + m:5 + m])
                        gl = stFt.tile([128, 272], F32, tag="cv_gl")
                        nc.vector.scalar_tensor_tensor(
                            out=gl[:], in0=za[:], scalar=bp1[:, m:m + 1],
                            in1=sg[:], op0=ALU.add, op1=ALU.mult)
                        nc.vector.tensor_mul(glu[:, m, sl], gl[:],
                                             cmb[:, sl])

            # depthwise conv: 31 accumulating diagonal matmuls per chunk
            acc = stF.tile([128, 4, HT], F32)
            with tc.tile_pool(name="dgp", bufs=2, space="PSUM") as dgp:
                for c in range(4):
                    psC = dgp.tile([128, HT], F32, tag="psC")
                    for j in range(KCONV):
                        nc.tensor.matmul(psC[:], dg[:, c, j, :],
                                         glu[:, c, 1 + j:1 + j + HT],
                                         start=(j == 0), stop=(j == KCONV - 1))
                    nc.vector.tensor_copy(acc[:, c, :], psC[:])
            # BN stats + 8-way AllReduce
            bnpack = stF.tile([128, 8], F32)
            for c in range(4):
                bst_t = stFt.tile([128, 6], F32, tag="bst")
                nc.vector.bn_stats(bst_t[:], acc[:, c, :])
                mv = stFt.tile([128, 2], F32, tag="mv")
                nc.vector.bn_aggr(mv[:], bst_t[:])
                nc.vector.tensor_copy(bnpack[:, 2 * c:2 * c + 1], mv[:, 0:1])
                nc.vector.scalar_tensor_tensor(
                    out=bnpack[:, 2 * c + 1:2 * c + 2], in0=mv[:, 0:1],
                    scalar=mv[:, 0:1], in1=mv[:, 1:2],
                    op0=ALU.mult, op1=ALU.add)
            nc.sync.dma_start(out=cc_bn_in[:], in_=bnpack[:])
            nc.gpsimd.collective_compute(
                "AllReduce", ALU.add, ins=[cc_bn_in[:]], outs=[cc_bn_out[:]],
                replica_groups=ALLG)
            with tc.tile_pool(name="pacps", bufs=1, space="PSUM") as pacps:
                pacer(pacps, stFt, 16, "bn")
            bnar = stF.tile([128, 8], F32)
            nc.sync.dma_start(out=bnar[:], in_=cc_bn_out[:])
            bng_sb = stF.tile([128, 4], F32)
            nc.sync.dma_start(out=bng_sb[:], in_=bng_d[:])
            bnb_sb = stF.tile([128, 4], F32)
            nc.sync.dma_start(out=bnb_sb[:], in_=bnb_d[:])
            ysl = stF.tile([128, 4, HT], BF16)
            for c in range(4):
                mg = stFt.tile([128, 1], F32, tag="mg")
                nc.scalar.activation(mg[:], bnar[:, 2 * c:2 * c + 1], AF.Copy,
                                     scale=1.0 / NCORES)
                e2 = stFt.tile([128, 1], F32, tag="e2c")
                nc.scalar.activation(e2[:], bnar[:, 2 * c + 1:2 * c + 2],
                                     AF.Copy, scale=1.0 / NCORES)
                vg = stFt.tile([128, 1], F32, tag="vg")
                nc.vector.tensor_mul(vg[:], mg[:], mg[:])
                nc.vector.tensor_sub(vg[:], e2[:], vg[:])
                sdc = stFt.tile([128, 1], F32, tag="sdc")
                nc.scalar.activation(sdc[:], vg[:], AF.Sqrt, bias=epsP[:])
                rs = stFt.tile([128, 1], F32, tag="rsc")
                nc.vector.reciprocal(rs[:], sdc[:])
                s1 = stFt.tile([128, 1], F32, tag="s1c")
                nc.vector.tensor_mul(s1[:], rs[:], bng_sb[:, c:c + 1])
                s2 = stFt.tile([128, 1], F32, tag="s2c")
                nc.vector.tensor_mul(s2[:], mg[:], s1[:])
                nc.vector.tensor_sub(s2[:], bnb_sb[:, c:c + 1], s2[:])
                sg2 = stFt.tile([128, HT], F32, tag="sg2")
                nc.scalar.activation(sg2[:], acc[:, c, :], AF.Sigmoid,
                                     scale=s1[:], bias=s2[:])
                yt = stFt.tile([128, HT], F32, tag="yt")
                nc.vector.tensor_scalar(yt[:], acc[:, c, :],
                                        s1[:], s2[:], ALU.mult, ALU.add)
                nc.vector.tensor_mul(ysl[:, c, :], yt[:], sg2[:])
            with tc.tile_pool(name="cvp2", bufs=2, space="PSUM") as cps2:
                for f in range(4):
                    psw = cps2.tile([128, HT], F32, tag="psw")
                    for c in range(4):
                        nc.tensor.matmul(psw[:], w2[:, c, ts(f, 128)],
                                         ysl[:, c, :],
                                         start=(c == 0), stop=(c == 3))
                    nc.vector.scalar_tensor_tensor(
                        out=h3_sb[:, f, :], in0=psw[:], scalar=bp2[:, f:f + 1],
                        in1=h2w[:, f, 16:16 + HT], op0=ALU.add, op1=ALU.add)
                    nc.vector.tensor_copy(h3_bf[:, f, :],
                                          h3_sb[:, f, :].bitcast(F32))

        # ================= Stage G: FFN2 =================
        with tc.tile_pool(name="stG", bufs=1) as stG, \
             tc.tile_pool(name="stGt", bufs=2) as stGt:
            rb4 = stG.tile([128, HT], F32)
            nb4 = stG.tile([128, HT], F32)
            emit_stats(h3_sb[:], 4, HT, [(0, HT)], rb4, nb4, 0, stGt,
                       _r(ones_c[:]))
            emit_ffn(h3_bf[:], h3_sb[:], rb4, nb4, h4_sb[:], None,
                     wg1_d, sg1_d, bg1_d, wg2_d, bg2_d, "f2")

        # ================= Stage H: LN5 + output =================
        with tc.tile_pool(name="stH", bufs=1) as stH, \
             tc.tile_pool(name="stHt", bufs=2) as stHt:
            g5_sb = stH.tile([128, 4], F32)
            nc.sync.dma_start(out=g5_sb[:], in_=g5_d[:])
            b5_sb = stH.tile([128, 4], F32)
            nc.sync.dma_start(out=b5_sb[:], in_=b5_d[:])
            rb5 = stH.tile([128, HT], F32)
            nb5 = stH.tile([128, HT], F32)
            emit_stats(h4_sb[:], 4, HT, [(0, HT)], rb5, nb5, 0, stHt,
                       _r(ones_c[:]))
            for c in range(4):
                u = stHt.tile([128, HT], F32, tag="h_u")
                nc.vector.tensor_mul(u[:], h4_sb[:, c, :], rb5[:])
                w_ = stHt.tile([128, HT], F32, tag="h_w")
                nc.vector.tensor_add(w_[:], u[:], nb5[:])
                xn5 = stHt.tile([128, HT], F32, tag="h_o")
                nc.vector.tensor_scalar(xn5[:], w_[:],
                                        g5_sb[:, c:c + 1], b5_sb[:, c:c + 1],
                                        ALU.mult, ALU.add)
                nc.sync.dma_start(out=out_d[c], in_=xn5[:])
    return nc


_CACHE = {}


def build_nc():
    if "nc" not in _CACHE:
        nc = bacc.Bacc("TRN2", target_bir_lowering=False, debug=False,
                       num_devices=NCORES)
        _emit(nc)
        nc.compile()
        _CACHE["nc"] = nc
    return _CACHE["nc"]


def _chunk_cf(a2d):
    """[Dany, W] -> [Dany//128, 128, W] chunk-major channels-first."""
    d, w = a2d.shape
    return np.ascontiguousarray(a2d.reshape(d // 128, 128, w), dtype=np.float32)


def to_bf16(a):
    import ml_dtypes
    return np.asarray(a, dtype=np.float32).astype(ml_dtypes.bfloat16)


def round_bf16(a):
    import ml_dtypes
    return np.asarray(a, dtype=np.float32).astype(
        ml_dtypes.bfloat16).astype(np.float32)


def _pcol(vec):
    """[Dout] per-channel vector -> [128, Dout//128] (partition, chunk)."""
    n = vec.shape[0]
    return np.ascontiguousarray(vec.reshape(n // 128, 128).T, dtype=np.float32)


def make_in_maps(inputs):
    inputs = {k: np.asarray(v, dtype=np.float32) for k, v in inputs.items()}
    x = inputs["x"]; pos_emb = inputs["pos_emb"]
    ln1_g, ln1_b = inputs["ln1_g"], inputs["ln1_b"]
    ln2_g, ln2_b = inputs["ln2_g"], inputs["ln2_b"]
    ln3_g, ln3_b = inputs["ln3_g"], inputs["ln3_b"]
    ln4_g, ln4_b = inputs["ln4_g"], inputs["ln4_b"]
    ln5_g, ln5_b = inputs["ln5_g"], inputs["ln5_b"]

    # FFN1/FFN2: W' = diag(g) W (bf16), b' = b + ln_b @ W, S = colsum(W')
    w1f = round_bf16(ln1_g[:, None] * inputs["ff1_w1"])
    b1f = inputs["ff1_b1"] + ln1_b @ inputs["ff1_w1"]
    s1f = w1f.sum(axis=0)
    w2f = round_bf16(0.5 * inputs["ff1_w2"]); b2f = 0.5 * inputs["ff1_b2"]
    wg1f = round_bf16(ln4_g[:, None] * inputs["ff2_w1"])
    bg1f = inputs["ff2_b1"] + ln4_b @ inputs["ff2_w1"]
    sg1f = wg1f.sum(axis=0)
    wg2f = round_bf16(0.5 * inputs["ff2_w2"]); bg2f = 0.5 * inputs["ff2_b2"]

    s = DK ** -0.5
    pos_u_f = inputs["pos_u"].reshape(D); pos_v_f = inputs["pos_v"].reshape(D)
    wqf = round_bf16(s * (ln2_g[:, None] * inputs["wq"]))
    bqf = s * (inputs["bq"] + ln2_b @ inputs["wq"] + pos_u_f)
    sqf = wqf.sum(axis=0)
    dqvf = s * (pos_v_f - pos_u_f)
    wkf = round_bf16(ln2_g[:, None] * inputs["wk"])
    bkf = inputs["bk"] + ln2_b @ inputs["wk"]
    skf = wkf.sum(axis=0)
    wvf = round_bf16(ln2_g[:, None] * inputs["wv"])
    bvf = inputs["bv"] + ln2_b @ inputs["wv"]
    svf = wvf.sum(axis=0)
    posT = np.zeros((D, 2048), dtype=np.float32)
    posT[:, :PB] = pos_emb[0].T

    pw1f = round_bf16((inputs["pw1_w"] * ln3_g[None, :]).T)    # [512, 1024]
    bpw1f = inputs["pw1_b"] + inputs["pw1_w"] @ ln3_b          # [1024]
    spwf = pw1f.sum(axis=0)
    dwwf = inputs["dw_w"][:, 0, :]                             # [512, 31]
    pw2f = round_bf16(inputs["pw2_w"].T)                       # [512, 512]

    base = {
        "wf1": to_bf16(_chunk_cf(w1f)), "sf1": _pcol(s1f), "bf1": _pcol(b1f),
        "wf2": to_bf16(_chunk_cf(w2f)), "bf2": _pcol(b2f),
        "wg1": to_bf16(_chunk_cf(wg1f)), "sg1": _pcol(sg1f),
        "bg1": _pcol(bg1f),
        "wg2": to_bf16(_chunk_cf(wg2f)), "bg2": _pcol(bg2f),
        "posT": to_bf16(_chunk_cf(posT)),
        "pw1": to_bf16(_chunk_cf(pw1f)), "spw": _pcol(spwf),
        "bpw1": _pcol(bpw1f),
        "dwv": np.ascontiguousarray(
            dwwf.reshape(4, 128, KCONV).transpose(1, 0, 2),
            dtype=np.float32),
        "bng": _pcol(inputs["bn_g"]), "bnb": _pcol(inputs["bn_b"]),
        "pw2": to_bf16(_chunk_cf(pw2f)), "bpw2": _pcol(inputs["pw2_b"]),
        "bo": _pcol(inputs["bo"]),
        "g5": _pcol(ln5_g), "b5": _pcol(ln5_b),
        "onc": np.ones((128, 1), dtype=np.float32),
        "onc_bf": to_bf16(np.ones((128, 1))),
        "idn": np.eye(128, dtype=np.float32),
    }

    in_maps = []
    for c in range(NCORES):
        b, scr = c // 2, c % 2
        cols = slice(256 * scr, 256 * scr + 256)
        m = dict(base)
        xb = x[b, scr * HT:(scr + 1) * HT, :].T               # [512, 512]
        m["x_loc"] = _chunk_cf(xb)
        m["x_bf"] = to_bf16(_chunk_cf(xb))
        m["wq"] = to_bf16(_chunk_cf(wqf[:, cols]))
        m["sq"] = _pcol(sqf[cols]); m["bq"] = _pcol(bqf[cols])
        m["dqv"] = _pcol(bqf[cols] + dqvf[cols])   # bqv = bq + dqv
        m["wk"] = to_bf16(_chunk_cf(wkf[:, cols]))
        m["sk"] = _pcol(skf[cols]); m["bk"] = _pcol(bkf[cols])
        m["wv"] = to_bf16(_chunk_cf(wvf[:, cols]))
        m["svrow"] = np.ascontiguousarray(svf[cols].reshape(1, 256),
                                          dtype=np.float32)
        m["bvrow"] = np.ascontiguousarray(bvf[cols].reshape(1, 256),
                                          dtype=np.float32)
        m["wp"] = to_bf16(_chunk_cf(inputs["wp"][:, cols]))
        wo_rows = inputs["wo"][cols, :]                       # [256, 512]
        m["wo"] = to_bf16(np.ascontiguousarray(wo_rows.reshape(4, 64, D)))
        cmask = np.ones((1, WIN), dtype=np.float32)
        if scr == 0:
            cmask[0, :16] = 0.0
        else:
            cmask[0, WIN - 16:] = 0.0
        m["cmask"] = cmask
        in_maps.append(m)
    return in_maps


def assemble_out(results):
    out = np.empty((B, T, D), dtype=np.float32)
    for c in range(NCORES):
        b, scr = c // 2, c % 2
        ol = np.asarray(results[c]["out_loc"])                # [4, 128, 512]
        out[b, scr * HT:(scr + 1) * HT, :] = ol.reshape(D, HT).T
    return out


def kernel(**inputs):
    in_maps = make_in_maps(inputs)
    nc = build_nc()
    res = run_bass_kernel_spmd(nc, in_maps, list(range(NCORES)))
    return assemble_out(res.results)
